# revision 12
# baseline (speedup 1.0000x reference)
"""Trainium2 Bass kernel for nn_DepPairingLayer (bidirectional chain-TreeLSTM over
shortest-path node chains + span mean-pooling + pair MLP), SPMD across 8 NeuronCores.

Sharding: data-parallel over the pair dimension P=8192 (1024 pairs/core); all
weights replicated.

The LSTM matmuls (both the x-projection and the h-recurrence) run in fp8e4m3
with DoubleRow perf mode: each matmul contracts 256 rows (two 128-row groups
packed as [128, 2, free] tiles), halving PE streaming time vs bf16. The full
contraction per gate tile is 5 DR blocks covering exactly
[x(832) | bias-row(1) | pad | U(384)] = 1280 rows: the x-tail block pairs the
last 64 x-rows + the folded bias row with h-block0 (copied into the node tile's
8th slot each step), and the last block pairs h-block1/h-block2. Weights are
pre-scaled x32 for fp8 range; the gate activation applies scale 1/32.
Gate activations read [128,2,512] 2-bank PSUM pairs in wide ACT instructions
(gate column order re-packed to f|u|i|o so sigmoid/tanh runs are contiguous).
The root-selection masks are precomputed once.

Up-direction early exit: pairs are host-sorted by root_idx within each batch,
and once a 256-block's active suffix narrows (width <= 248) every engine --
PE matmuls, gate ACT, DVE state math, h copies, root captures -- operates on
exactly the two active segments, so no tile carries a partially-stale region.
At s==0 the f-gate m-pair is skipped entirely (c_prev == 0). All gate
activations are emitted in j-order so the PSUM ring recycles at ACT cadence.
The span-feature half of the MLP first layer (12 of 21 k-tiles) is contracted
during phase 1 per chunk as soon as its two batches finish pooling (PSUM and
PE slack exist there) and re-added via DVE in phase 3, shortening the serial
MLP tail. Everything else (span pooling, captures, MLP) stays bf16; measured
end-to-end rel-absmax error vs the fp32 reference: ~9e-3 (threshold 2e-2).
"""

from contextlib import ExitStack

import numpy as np
import ml_dtypes

import concourse.bass as bass
import concourse.mybir as mybir
import concourse.tile as tile
from concourse import bacc
from concourse.bass_utils import run_bass_kernel_spmd
from concourse.masks import make_identity

bf16 = ml_dtypes.bfloat16
f8e4 = ml_dtypes.float8_e4m3
FP32 = mybir.dt.float32
BF16 = mybir.dt.bfloat16
F8 = mybir.dt.float8e4
FP16 = mybir.dt.float16
U8 = mybir.dt.uint8
ALU = mybir.AluOpType
ACTF = mybir.ActivationFunctionType
DR = mybir.MatmulPerfMode.DoubleRow

# problem dims (hardcoded per contract)
NCORES = 8
B, PB, L, D, H, DT, T = 32, 256, 16, 832, 384, 768, 512
P = B * PB                      # 8192 pairs
PS = P // NCORES                # 1024 pairs per core
NB = B // NCORES                # 4 batches per core
C = 512                         # pair-chunk (matmul moving free dim)
NCH = PS // C                   # 2 chunks per core
H4 = 4 * H                      # 1536 gate features, column order i|o|f|u
M12 = H4 // 128                 # 12 m-tiles of gate features
NP6 = M12 // 2                  # 6 m-pairs (one [128,1024] PSUM pair each)
KH = H // 128                   # 3 k-subtiles of hidden
NKB = 5                         # DR blocks: 3x pure-x, x-tail|U0, U1|U2
WSC = 32.0                      # fp8 weight pre-scale
DEC_IN, DEC_H, DEC_OUT = 3 * H + 2 * DT, 512, 7
K21 = DEC_IN // 128             # 21 feature k-tiles for W1
M4 = DEC_H // 128               # 4 m-tiles for W1 output
MT = DT // 128                  # 6 span-feature m-tiles
JT = PB // 128                  # 2 pair-tiles per batch (for masks)


def _build_program(debug: bool = False, loop_n: int = 0,
                   widths=None) -> bass.Bass:
    """loop_n > 0 wraps the whole body in a For_i loop executing it loop_n
    times (identical work each iteration) — used only for timing via
    (T(N) - T(1)) / (N - 1).

    widths[s] = active pair-column count per 256-block at up-dir step s
    (pairs are host-sorted by root_idx within each batch, so up-direction
    matmuls only touch the suffix window that still has unreached roots).
    None -> full width. Defaults to the widths stashed by _prep_in_maps."""
    if widths is None:
        widths = _CACHE.get("widths", (256,) * L)
    nc = bacc.Bacc("TRN2", target_bir_lowering=False, debug=False,
                   num_devices=NCORES)
    dp = nc.declare_dram_parameter
    if debug:
        dbg_span = dp("dbg_span", [2, MT, 128, PS], BF16, isOutput=True)
        dbg_racc = dp("dbg_racc", [NCH, KH, 128, C], BF16, isOutput=True)
        dbg_start = dp("dbg_start", [NCH, 128, KH, C], BF16, isOutput=True)
        dbg_end = dp("dbg_end", [NCH, 128, KH, C], BF16, isOutput=True)

    node_dr = dp("node_dr", [L, 128, 7, PS], F8, isOutput=False)
    tok = dp("tok", [NB, T, DT], BF16, isOutput=False)
    spb = dp("spb", [2, NB, 3 * PB], FP16, isOutput=False)
    mask8_d = dp("mask8_d", [L, 128, PS], U8, isOutput=False)
    Wu = dp("Wu", [128, NKB, 2, H4], F8, isOutput=False)
    Wd = dp("Wd", [128, NKB, 2, H4], F8, isOutput=False)
    W1 = dp("W1", [DEC_IN, DEC_H], BF16, isOutput=False)
    W2 = dp("W2", [DEC_H, DEC_OUT], BF16, isOutput=False)
    b1 = dp("b1", [M4, 128, 1], FP32, isOutput=False)
    b2 = dp("b2", [DEC_OUT, 1], FP32, isOutput=False)
    ones = dp("ones", [1, 128], FP16, isOutput=False)
    iota_c = dp("iota_c", [128, T // 128], FP32, isOutput=False)
    out_d = dp("out", [DEC_OUT, PS], FP32, isOutput=True)

    def loadc(pool, name, src_ap, shape, dtype, bufs=1):
        t = pool.tile(shape, dtype, name=name, tag=name, bufs=bufs)
        nc.sync.dma_start(t[:], src_ap)
        return t

    with tile.TileContext(nc) as tc, ExitStack() as ctx:
        if loop_n:
            ctx.enter_context(tc.For_i(0, loop_n, 1))
        # whole-program pools
        cpool = ctx.enter_context(tc.tile_pool(name="const", bufs=1))
        spanp = ctx.enter_context(tc.tile_pool(name="spanp", bufs=1))
        capp = ctx.enter_context(tc.tile_pool(name="capp", bufs=1))

        # spanT[sp][m]: [128, PS] bf16 feature-major span means (whole program)
        spanT = [[spanp.tile([128, PS], BF16, name=f"span{sp}_{m}",
                             tag=f"span{sp}_{m}") for m in range(MT)]
                 for sp in range(2)]
        # span-feature partial of the MLP first layer, computed in phase 1
        # (PSUM free there) and re-added in phase 3: [128, C] bf16 per (m, ch)
        z_span = [[spanp.tile([128, C], BF16, name=f"zs{m}_{ch}",
                              tag=f"zs{m}_{ch}") for ch in range(NCH)]
                  for m in range(M4)]
        # per-chunk LSTM summary tiles (whole program; consumed by the MLP)
        root_acc = [[capp.tile([128, C], BF16, name=f"racc{ch}_{k}",
                               tag=f"racc{ch}_{k}") for k in range(KH)]
                    for ch in range(NCH)]
        start_t = [None] * NCH
        end_t = [None] * NCH

        # ---- phase 1: span mean pooling --------------------------------
        # span masks are built directly in transposed [token(part), pair]
        # layout: st/en/recip are PE-broadcast across partitions (fp16 ones
        # outer product), then compared against a per-partition token iota.
        # The recip scale is applied after the pooling matmul.
        with tc.tile_pool(name="tokp", bufs=2) as tokp, \
             tc.tile_pool(name="mwork", bufs=2) as mwork, \
             tc.tile_pool(name="spsum", bufs=4, space="PSUM") as spsum:
            iota_ct = loadc(tokp, "iotac", iota_c[:, :], [128, T // 128], FP32)

            def load_tok(b):
                tk = []
                for tb in range(T // 128):
                    t = tokp.tile([128, DT], BF16, name=f"tok{tb}",
                                  tag=f"tok{tb}")
                    nc.sync.dma_start(t[:], tok[b, tb * 128:(tb + 1) * 128, :])
                    tk.append(t)
                return tk

            tk0 = load_tok(0)
            w1s_t = [loadc(mwork, f"w1s{k}", W1[(9 + k) * 128:(10 + k) * 128, :],
                           [128, DEC_H], BF16) for k in range(2 * MT)]
            # heavy constant DMAs on the gpsimd DGE queue: they run in
            # parallel with the phase-1-critical SP-queue loads above
            b1_t = [loadc(cpool, f"b1{m}", b1[m], [128, 1], FP32)
                    for m in range(M4)]
            b2_t = loadc(cpool, "b2t", b2[:, :], [DEC_OUT, 1], FP32)
            ones_t = cpool.tile([1, 128], FP16, name="onest", tag="onest")
            nc.gpsimd.dma_start(ones_t[:], ones[:, :])
            w_t = {}
            for d, W in (("u", Wu), ("d", Wd)):
                w_t[d] = cpool.tile([128, NKB, 2, H4], F8, name=f"wdr{d}",
                                    tag=f"wdr{d}")
                nc.gpsimd.dma_start(w_t[d][:], W[:, :, :, :])
            mask8 = []
            for s in range(L):
                m8 = capp.tile([128, PS], U8, name=f"mask{s}", tag=f"mask{s}")
                nc.gpsimd.dma_start(m8[:], mask8_d[s])
                mask8.append(m8)
            for b in range(NB):
                tk = tk0 if b == 0 else load_tok(b)
                for sp in range(2):
                    spt = mwork.tile([1, 3 * PB], FP16, name="spt", tag="spt",
                                     bufs=4)
                    nc.sync.dma_start(spt[:], spb[sp, b])
                    bc_ps = spsum.tile([128, 3 * PB], FP32, name="bc",
                                       tag="bc", bufs=2)
                    for r in range(3):
                        nc.tensor.matmul(bc_ps[:, r * PB:(r + 1) * PB],
                                         ones_t[:],
                                         spt[:, r * PB:(r + 1) * PB],
                                         start=True, stop=True)
                    bcs = mwork.tile([128, 3, PB], FP32, name="bcs", tag="bcs")
                    nc.vector.tensor_copy(bcs[:], bc_ps[:])
                    maskT = [mwork.tile([128, PB], BF16, name=f"mT{tb}",
                                        tag=f"mT{tb}") for tb in range(T // 128)]
                    for tb in range(T // 128):
                        c1 = mwork.tile([128, PB], BF16, name="c1", tag="c1",
                                        bufs=4)
                        c2 = mwork.tile([128, PB], BF16, name="c2", tag="c2",
                                        bufs=4)
                        nc.vector.tensor_scalar(c1[:], bcs[:, 0, :],
                                                iota_ct[:, tb:tb + 1], None,
                                                ALU.is_le)
                        nc.vector.tensor_scalar(c2[:], bcs[:, 1, :],
                                                iota_ct[:, tb:tb + 1], None,
                                                ALU.is_gt)
                        nc.gpsimd.tensor_tensor(maskT[tb][:], c1[:], c2[:],
                                                ALU.mult)
                    for m in range(MT):
                        zp = spsum.tile([128, PB], FP32, name="zp", tag="mm")
                        for tb in range(T // 128):
                            nc.tensor.matmul(zp[:], tk[tb][:, m * 128:(m + 1) * 128],
                                             maskT[tb][:], start=(tb == 0),
                                             stop=(tb == T // 128 - 1))
                        nc.vector.tensor_tensor(
                            spanT[sp][m][:, b * PB:(b + 1) * PB], zp[:],
                            bcs[:, 2, :], ALU.mult)
                # span-feature partial z for chunk ch = b//2: 12 of the 21 W1
                # k-tiles contract here, filling PE slack off the phase-3 tail
                if b % 2 == 1:
                    ch = b // 2
                    c0 = ch * C
                    sfeats = ([spanT[0][m][:, c0:c0 + C] for m in range(MT)]
                              + [spanT[1][m][:, c0:c0 + C] for m in range(MT)])
                    for m in range(M4):
                        zps = spsum.tile([128, C], FP32, name="zps", tag="mm")
                        for k in range(2 * MT):
                            nc.tensor.matmul(zps[:],
                                             w1s_t[k][:, m * 128:(m + 1) * 128],
                                             sfeats[k], start=(k == 0),
                                             stop=(k == 2 * MT - 1))
                        nc.vector.tensor_copy(z_span[m][ch][:], zps[:])

        # ---- phase 2: bidirectional chain-LSTM, fp8 DoubleRow ----------
        with tc.tile_pool(name="nodep", bufs=2) as nodep, \
             tc.tile_pool(name="cstp", bufs=2) as cstp, \
             tc.tile_pool(name="hdrp", bufs=2) as hdrp, \
             tc.tile_pool(name="gatep", bufs=4) as gatep, \
             tc.tile_pool(name="scrp", bufs=2) as scrp, \
             tc.tile_pool(name="pmm", bufs=2, space="PSUM") as pmm:
            for ch in range(NCH):
                for k in range(KH):
                    nc.vector.memset(root_acc[ch][k][:], 0.0)

            def new_node_tile(d, ch, t_src, memset7):
                t = nodep.tile([128, 8, C], F8, name=f"nd_{d}{ch}",
                               tag=f"nd_{d}{ch}")
                nc.sync.dma_start(t[:, 0:7, :],
                                  node_dr[t_src, :, :, ch * C:(ch + 1) * C])
                if memset7:
                    nc.vector.memset(t[:, 7:8, :], 0.0)
                return t

            nd_cur = {}
            for d in ("u", "d"):
                for ch in range(NCH):
                    nd_cur[d, ch] = new_node_tile(
                        d, ch, 0 if d == "u" else L - 1, True)
            cst = {}
            hdr = {}

            for s in range(L):
                for d in ("u", "d"):
                    nd_nx = {}
                    if s + 1 < L:
                        t_src = (s + 1) if d == "u" else L - 2 - s
                        for ch in range(NCH):
                            nd_nx[ch] = new_node_tile(d, ch, t_src, False)

                    # -- gate matmuls: 6 m-pairs, software-pipelined so the
                    # h-dependent blocks (kb3/kb4) of pair j are emitted after
                    # the x-only blocks (kb0..2) of pair j+1.
                    pm = {}

                    # at s==0 the f-gates are unused (c_prev == 0): skip m
                    # pair 0 (f0,f1) entirely.
                    if s == 0:
                        halves = {0: (), 1: (0, 1), 2: (0, 1), 3: (0, 1),
                                  4: (0, 1), 5: (0, 1)}
                    else:
                        halves = {j: (0, 1) for j in range(NP6)}

                    # up-dir active window: pairs sorted by root within
                    # each 256-block; only the suffix [o, 256) of each block
                    # still needs the recurrence at step s. Every engine
                    # (PE, ACT, DVE, Pool, captures) touches exactly these
                    # segments, so no tile ever carries a partially-stale
                    # region that a later full-width op would read.
                    if d == "u" and widths[s] <= 248:
                        o = 256 - widths[s]
                        op_segs = [(o, widths[s]), (256 + o, widths[s])]
                    else:
                        op_segs = [(0, 512)]
                    mm_segs = op_segs
                    nsg_mm = len(mm_segs)

                    def emit_x(j):
                        if not halves[j]:
                            return
                        pm[j] = pmm.tile([128, 2, 2, C], FP32, name="pm",
                                         tag="mm")
                        for half in halves[j]:
                            m = 2 * j + half
                            for kb in range(3):
                                for ch in range(NCH):
                                    for sg, (off, w) in enumerate(mm_segs):
                                        nc.tensor.matmul(
                                            pm[j][:, ch, half, off:off + w],
                                            w_t[d][:, kb, :,
                                                   m * 128:(m + 1) * 128],
                                            nd_cur[d, ch][:, 2 * kb:2 * kb + 2,
                                                          off:off + w],
                                            start=(kb == 0 and sg == 0),
                                            stop=False, perf_mode=DR)

                    def emit_h(j):
                        for half in halves[j]:
                            m = 2 * j + half
                            for ch in range(NCH):
                                for sg, (off, w) in enumerate(mm_segs):
                                    nc.tensor.matmul(
                                        pm[j][:, ch, half, off:off + w],
                                        w_t[d][:, 3, :, m * 128:(m + 1) * 128],
                                        nd_cur[d, ch][:, 6:8, off:off + w],
                                        start=False,
                                        stop=(s == 0 and sg == nsg_mm - 1),
                                        perf_mode=DR)
                            if s > 0:
                                for ch in range(NCH):
                                    for sg, (off, w) in enumerate(mm_segs):
                                        nc.tensor.matmul(
                                            pm[j][:, ch, half, off:off + w],
                                            w_t[d][:, 4, :,
                                                   m * 128:(m + 1) * 128],
                                            hdr[d, ch][:, :, off:off + w],
                                            start=False,
                                            stop=(sg == nsg_mm - 1),
                                            perf_mode=DR)

                    gates = {}

                    def emit_act(j):
                        # gate column order f|u|i|o: pairs 0,3,4,5 sigmoid,
                        # pair 1 = (f2|u0) mixed, pair 2 = (u1,u2) tanh.
                        # One wide instr covers both chunks; reads cover
                        # exactly the matmul-written segments.
                        if not halves[j]:
                            gates[j] = None
                            return
                        g = gatep.tile([128, 2, 2, C], BF16, name="g", tag="g")
                        src = pm[j]
                        if j == 1:
                            for off, w in op_segs:
                                nc.scalar.activation(g[:, :, 0, off:off + w],
                                                     src[:, :, 0, off:off + w],
                                                     ACTF.Sigmoid,
                                                     scale=1.0 / WSC)
                                nc.scalar.activation(g[:, :, 1, off:off + w],
                                                     src[:, :, 1, off:off + w],
                                                     ACTF.Tanh,
                                                     scale=1.0 / WSC)
                        else:
                            fn = ACTF.Tanh if j == 2 else ACTF.Sigmoid
                            for off, w in op_segs:
                                nc.scalar.activation(g[:, :, :, off:off + w],
                                                     src[:, :, :, off:off + w],
                                                     fn, scale=1.0 / WSC)
                        gates[j] = g

                    emit_x(0)
                    for j in range(NP6):
                        if j + 1 < NP6:
                            emit_x(j + 1)
                        emit_h(j)
                        emit_act(j)

                    # -- state update per chunk, window-segmented. All gate
                    # activations are queued in j-order above, so the PSUM
                    # pool recycles at gate-ACT cadence and the next
                    # step-dir's x-matmuls never wait on this (DVE/ACT) tail.
                    hb0 = {}
                    hb12 = {}
                    tcs = {}
                    for ch in range(NCH):
                        gs = {j: (g[:, ch] if g is not None else None)
                              for j, g in gates.items()}
                        u_ = [gs[1][:, 1, :], gs[2][:, 0, :], gs[2][:, 1, :]]
                        i_ = [gs[3][:, 0, :], gs[3][:, 1, :], gs[4][:, 0, :]]
                        cn = cstp.tile([128, KH, C], BF16, name=f"c_{d}{ch}",
                                       tag=f"c_{d}{ch}")
                        if s == 0:
                            for k in range(KH):
                                for off, w in op_segs:
                                    nc.vector.tensor_tensor(
                                        cn[:, k, off:off + w],
                                        i_[k][:, off:off + w],
                                        u_[k][:, off:off + w], ALU.mult)
                        else:
                            f2 = gs[1][:, 0, :]
                            tmp = scrp.tile([128, KH, C], BF16, name="tmp",
                                            tag="tmp", bufs=2)
                            for k in range(KH):
                                for off, w in op_segs:
                                    nc.vector.tensor_tensor(
                                        tmp[:, k, off:off + w],
                                        i_[k][:, off:off + w],
                                        u_[k][:, off:off + w], ALU.mult)
                            for off, w in op_segs:
                                nc.vector.tensor_tensor(
                                    cn[:, 0:2, off:off + w],
                                    gs[0][:, :, off:off + w],
                                    cst[d, ch][:, 0:2, off:off + w], ALU.mult)
                                nc.vector.tensor_tensor(
                                    cn[:, 2, off:off + w],
                                    f2[:, off:off + w],
                                    cst[d, ch][:, 2, off:off + w], ALU.mult)
                                nc.vector.tensor_tensor(
                                    cn[:, :, off:off + w],
                                    cn[:, :, off:off + w],
                                    tmp[:, :, off:off + w], ALU.add)
                        cst[d, ch] = cn
                        tc_ = scrp.tile([128, KH, C], BF16, name="tc", tag="tc",
                                        bufs=3)
                        for off, w in op_segs:
                            nc.scalar.activation(tc_[:, :, off:off + w],
                                                 cn[:, :, off:off + w],
                                                 ACTF.Tanh)
                        tcs[ch] = tc_
                        h0 = scrp.tile([128, C], BF16, name="hb0", tag="hb0",
                                       bufs=3)
                        for off, w in op_segs:
                            nc.vector.tensor_tensor(h0[:, off:off + w],
                                                    gs[4][:, 1, off:off + w],
                                                    tc_[:, 0, off:off + w],
                                                    ALU.mult)
                        hb0[ch] = h0
                        if s + 1 < L:
                            for off, w in op_segs:
                                nc.vector.tensor_copy(
                                    nd_nx[ch][:, 7:8, off:off + w],
                                    h0[:, off:off + w])
                    for ch in range(NCH):
                        g5 = gates[5][:, ch]
                        h12 = scrp.tile([128, 2, C], BF16, name="hb12",
                                        tag="hb12", bufs=3)
                        for off, w in op_segs:
                            nc.vector.tensor_tensor(
                                h12[:, :, off:off + w],
                                g5[:, :, off:off + w],
                                tcs[ch][:, 1:KH, off:off + w], ALU.mult)
                        hb12[ch] = h12
                        if s + 1 < L:
                            hd = hdrp.tile([128, 2, C], F8, name=f"h_{d}{ch}",
                                           tag=f"h_{d}{ch}")
                            for off, w in op_segs:
                                nc.gpsimd.tensor_copy(hd[:, :, off:off + w],
                                                      h12[:, :, off:off + w])
                            hdr[d, ch] = hd
                    for ch in range(NCH):
                        if d == "u":
                            for off, w in op_segs:
                                mseg = mask8[s][:, ch * C + off:
                                                ch * C + off + w]
                                nc.vector.copy_predicated(
                                    root_acc[ch][0][:, off:off + w], mseg,
                                    hb0[ch][:, off:off + w])
                                for k in range(1, KH):
                                    nc.vector.copy_predicated(
                                        root_acc[ch][k][:, off:off + w], mseg,
                                        hb12[ch][:, k - 1, off:off + w])
                        else:
                            if s == 0:
                                end_t[ch] = capp.tile([128, KH, C], BF16,
                                                      name=f"end{ch}",
                                                      tag=f"end{ch}")
                                nc.gpsimd.tensor_copy(end_t[ch][:, 0, :],
                                                      hb0[ch][:])
                                nc.gpsimd.tensor_copy(end_t[ch][:, 1:KH, :],
                                                      hb12[ch][:])
                            if s == L - 1:
                                start_t[ch] = capp.tile([128, KH, C], BF16,
                                                        name=f"start{ch}",
                                                        tag=f"start{ch}")
                                nc.gpsimd.tensor_copy(start_t[ch][:, 0, :],
                                                      hb0[ch][:])
                                nc.gpsimd.tensor_copy(start_t[ch][:, 1:KH, :],
                                                      hb12[ch][:])
                    if s + 1 < L:
                        for ch in range(NCH):
                            nd_cur[d, ch] = nd_nx[ch]

        if debug:
            for sp in range(2):
                for m in range(MT):
                    nc.sync.dma_start(dbg_span[sp, m], spanT[sp][m][:])
            for ch in range(NCH):
                for k in range(KH):
                    nc.sync.dma_start(dbg_racc[ch, k], root_acc[ch][k][:])
                nc.sync.dma_start(dbg_start[ch], start_t[ch][:])
                nc.sync.dma_start(dbg_end[ch], end_t[ch][:])

        # ---- phase 3: pair MLP -----------------------------------------
        with tc.tile_pool(name="mlpw", bufs=1) as mlpw, \
             tc.tile_pool(name="mlpp", bufs=4) as mlpp, \
             tc.tile_pool(name="mpsum", bufs=6, space="PSUM") as mpsum, \
             tc.tile_pool(name="pout", bufs=1, space="PSUM") as pout:
            w1_t = [loadc(mlpw, f"w1{k}", W1[k * 128:(k + 1) * 128, :],
                          [128, DEC_H], BF16) for k in range(3 * KH)]
            w2_t = [loadc(mlpw, f"w2{k}", W2[k * 128:(k + 1) * 128, :],
                          [128, DEC_OUT], BF16) for k in range(M4)]
            for ch in range(NCH):
                c0 = ch * C
                feats = ([root_acc[ch][k][:] for k in range(KH)]
                         + [start_t[ch][:, k, :] for k in range(KH)]
                         + [end_t[ch][:, k, :] for k in range(KH)])
                z_t = []
                for m in range(M4):
                    zp = mpsum.tile([128, C], FP32, name="zp2", tag="mm")
                    for k in range(3 * KH):
                        nc.tensor.matmul(zp[:], w1_t[k][:, m * 128:(m + 1) * 128],
                                         feats[k], start=(k == 0),
                                         stop=(k == 3 * KH - 1))
                    zs = mlpp.tile([128, C], BF16, name="zs", tag="zsum")
                    nc.vector.tensor_tensor(zs[:], zp[:], z_span[m][ch][:],
                                            ALU.add)
                    z = mlpp.tile([128, C], BF16, name="z", tag="z")
                    nc.scalar.activation(z[:], zs[:], ACTF.Tanh, bias=b1_t[m][:])
                    z_t.append(z)
                op = pout.tile([DEC_OUT, C], FP32, name="op", tag="op")
                for m in range(M4):
                    nc.tensor.matmul(op[:], w2_t[m][:], z_t[m][:], start=(m == 0),
                                     stop=(m == M4 - 1))
                osb = mlpp.tile([DEC_OUT, C], FP32, name="osb", tag="osb", bufs=2)
                nc.vector.tensor_scalar(osb[:], op[:], b2_t[:], None, ALU.add)
                nc.sync.dma_start(out_d[:, c0:c0 + C], osb[:])

    nc.compile()
    _dedupe_ldweights(nc)
    return nc


def _dedupe_ldweights(nc):
    """Remove PE InstLdweights whose weights AP equals the most recently
    retained one with only PE Matmults in between (the PE weight buffer is
    unchanged by other engines). Only wait-free/update-free loads are removed."""
    import concourse.mybir as _mb
    for name, bb in list(nc.bb_map.items()):
        insts = bb.bb.instructions
        out = []
        prev_sig = None
        removed = 0
        for inst in insts:
            tn = type(inst).__name__
            eng = getattr(inst, "engine", None)
            if eng == _mb.EngineType.PE:
                if tn == "InstLdweights":
                    si = inst.sync_info
                    clean = si is None or (not si.on_wait and not si.on_update)
                    try:
                        sig = str(inst.ins[0])
                    except Exception:
                        sig = None
                    if clean and sig is not None and sig == prev_sig:
                        removed += 1
                        continue
                    prev_sig = sig
                elif tn != "InstMatmult":
                    prev_sig = None
            out.append(inst)
        if removed:
            bb.bb.instructions = out


_CACHE = {}


def _get_program() -> bass.Bass:
    if "nc" not in _CACHE:
        _CACHE["nc"] = _build_program()
    return _CACHE["nc"]


def _prep_in_maps(inputs) -> list[dict]:
    f32 = np.float32
    node = np.asarray(inputs["node_embs"], f32)
    tokf = np.asarray(inputs["token_embs"], f32)
    rooti = np.asarray(inputs["root_idx"])

    # sort pairs by root_idx within each batch (pure layout permutation,
    # inverted on the output): enables the up-dir early-exit windows
    rb = rooti.reshape(B, PB)
    order = np.argsort(rb, axis=1, kind="stable")           # [B, PB]
    perm_full = (order + (np.arange(B) * PB)[:, None]).reshape(P)
    _CACHE["inv_perm"] = np.argsort(perm_full)
    node = node[perm_full]
    rooti = rooti[perm_full]
    rb_s = rooti.reshape(B, PB)
    # widths[s] = 256 - min over batches of #(root < s); exact cover for
    # every batch block on every core
    cnt = (rb_s[:, :, None] < np.arange(L)[None, None, :]).sum(1)   # [B, L]
    _CACHE["widths"] = tuple(int(256 - c) for c in cnt.min(0))

    def bperm(a):
        return np.take_along_axis(np.asarray(a), order, axis=1)

    inputs = dict(inputs,
                  p1_st=bperm(inputs["p1_st"]), p1_len=bperm(inputs["p1_len"]),
                  p2_st=bperm(inputs["p2_st"]), p2_len=bperm(inputs["p2_len"]))
    # [P, L, D] fp32 -> per-core [L, 128, 7, PS] fp8 with bias row appended:
    # rows 0..831 = x, row 832 = 1.0, rows 833..895 = 0 (7 k-subtiles of 128)
    n8 = node.astype(f8e4).reshape(NCORES, PS, L, D).transpose(0, 2, 3, 1)
    pad = np.zeros((NCORES, L, 896, PS), f8e4)
    pad[:, :, :D, :] = n8
    pad[:, :, D, :] = f8e4(1.0)
    node_sh = np.ascontiguousarray(
        pad.reshape(NCORES, L, 7, 128, PS).transpose(0, 1, 3, 2, 4))
    tok_sh = tokf.reshape(NCORES, NB, T, DT).astype(bf16)

    def span_arrays(st, ln):
        st = np.asarray(st).astype(f32)
        ln = np.asarray(ln).astype(f32)
        en = st + ln + 1.0
        rc = 1.0 / (ln + 1.0)
        return st, en, rc

    s1, e1, r1 = span_arrays(inputs["p1_st"], inputs["p1_len"])
    s2, e2, r2 = span_arrays(inputs["p2_st"], inputs["p2_len"])

    def pack_span(a1, a2):
        # [B, PB] x2 -> per-core [2, NB, PB]
        a = np.stack([a1, a2])  # [2, B, PB]
        return a.reshape(2, NCORES, NB, PB).transpose(1, 0, 2, 3)

    # [NCORES, 2, NB, 3, PB] fp16 rows: st | en | recip (values <= 503 are
    # exact in fp16; recip has 10-bit mantissa)
    spb = np.ascontiguousarray(np.stack(
        [pack_span(s1, s2), pack_span(e1, e2), pack_span(r1, r2)],
        axis=3).astype(np.float16)).reshape(NCORES, 2, NB, 3 * PB)

    # one-hot root masks, broadcast across partitions: [NCORES, L, 128, PS]
    oh = (rooti.reshape(NCORES, 1, PS) ==
          np.arange(L, dtype=rooti.dtype).reshape(1, L, 1))
    mask8_h = np.ascontiguousarray(np.broadcast_to(
        oh[:, :, None, :], (NCORES, L, 128, PS)).astype(np.uint8))

    # gate-column permutation i|o|u|f -> f|u|i|o
    perm = np.concatenate([np.arange(3 * H, 4 * H), np.arange(2 * H, 3 * H),
                           np.arange(0, H), np.arange(H, 2 * H)])

    def build_wdr(Wiou, Wf, Uiou, Uf, biou, bf):
        W = np.concatenate([np.asarray(Wiou, f32), np.asarray(Wf, f32)],
                           axis=1)[:, perm]      # [D, 4H]
        U = np.concatenate([np.asarray(Uiou, f32), np.asarray(Uf, f32)],
                           axis=1)[:, perm]      # [H, 4H]
        b = np.concatenate([np.asarray(biou, f32), np.asarray(bf, f32)])[perm]
        Wpad = np.zeros((NKB * 256, H4), f32)
        Wpad[:D] = W
        Wpad[D] = b
        Wpad[896:896 + H] = U
        W8 = (Wpad * WSC).astype(f8e4)
        # [5 kb, 2 grp, 128 p, H4] -> [128, 5, 2, H4]
        return np.ascontiguousarray(
            W8.reshape(NKB, 2, 128, H4).transpose(2, 0, 1, 3))

    Wu_h = build_wdr(inputs["Wiou_u"], inputs["Wf_u"], inputs["Uiou_u"],
                     inputs["Uf_u"], inputs["biou_u"], inputs["bf_u"])
    Wd_h = build_wdr(inputs["Wiou_d"], inputs["Wf_d"], inputs["Uiou_d"],
                     inputs["Uf_d"], inputs["biou_d"], inputs["bf_d"])
    W1_h = np.asarray(inputs["W1"], f32).astype(bf16)
    W2_h = np.asarray(inputs["W2"], f32).astype(bf16)
    b1_h = np.asarray(inputs["b1"], f32).reshape(M4, 128, 1)
    b2_h = np.asarray(inputs["b2"], f32).reshape(DEC_OUT, 1)
    ones_h = np.ones((1, 128), np.float16)
    iota_h = np.ascontiguousarray(
        (np.arange(T // 128, dtype=f32)[None, :] * 128
         + np.arange(128, dtype=f32)[:, None]))

    in_maps = []
    for c in range(NCORES):
        in_maps.append({
            "node_dr": node_sh[c], "tok": tok_sh[c],
            "spb": spb[c], "mask8_d": mask8_h[c],
            "Wu": Wu_h, "Wd": Wd_h,
            "W1": W1_h, "W2": W2_h,
            "b1": b1_h, "b2": b2_h, "ones": ones_h, "iota_c": iota_h,
        })
    return in_maps


def unpermute(out_full: np.ndarray) -> np.ndarray:
    """Invert the host-side pair sort on a full [P, ...] output."""
    return out_full[_CACHE["inv_perm"]]


def run(inputs, **kwargs):
    """Run on hardware; returns (output [P, DEC_OUT] fp32, BassKernelResults)."""
    in_maps = _prep_in_maps(inputs)
    nc = _get_program()
    res = run_bass_kernel_spmd(nc, in_maps, list(range(NCORES)), **kwargs)
    outs = [np.asarray(r["out"], np.float32).T for r in res.results]  # [PS, 7]
    return unpermute(np.concatenate(outs, axis=0)), res


def kernel(**inputs) -> np.ndarray:
    out, _ = run(inputs)
    return out



# revision 13
# speedup vs baseline: 1.0579x; 1.0579x over previous
"""Trainium2 Bass kernel for nn_DepPairingLayer (bidirectional chain-TreeLSTM over
shortest-path node chains + span mean-pooling + pair MLP), SPMD across 8 NeuronCores.

Sharding: data-parallel over the pair dimension P=8192 (1024 pairs/core); all
weights replicated.

The LSTM matmuls (both the x-projection and the h-recurrence) run in fp8e4m3
with DoubleRow perf mode: each matmul contracts 256 rows (two 128-row groups
packed as [128, 2, free] tiles), halving PE streaming time vs bf16. The full
contraction per gate tile is 5 DR blocks covering exactly
[x(832) | bias-row(1) | pad | U(384)] = 1280 rows: the x-tail block pairs the
last 64 x-rows + the folded bias row with h-block0 (copied into the node tile's
8th slot each step), and the last block pairs h-block1/h-block2. Weights are
pre-scaled x32 for fp8 range; the gate activation applies scale 1/32.
Gate activations read [128,2,512] 2-bank PSUM pairs in wide ACT instructions
(gate column order re-packed to f|u|i|o so sigmoid/tanh runs are contiguous).
The root-selection masks are precomputed once.

Up-direction early exit: pairs are host-sorted by root_idx within each batch,
and once a 256-block's active suffix narrows (width <= 248) every engine --
PE matmuls, gate ACT, DVE state math, h copies, root captures -- operates on
exactly the two active segments, so no tile carries a partially-stale region.
At s==0 the f-gate m-pair is skipped entirely (c_prev == 0). All gate
activations are emitted in j-order so the PSUM ring recycles at ACT cadence.
The span-feature half of the MLP first layer (12 of 21 k-tiles) is contracted
during phase 1 per chunk as soon as its two batches finish pooling (PSUM and
PE slack exist there) and re-added via DVE in phase 3, shortening the serial
MLP tail. Everything else (span pooling, captures, MLP) stays bf16; measured
end-to-end rel-absmax error vs the fp32 reference: ~9e-3 (threshold 2e-2).
"""

from contextlib import ExitStack

import numpy as np
import ml_dtypes

import concourse.bass as bass
import concourse.mybir as mybir
import concourse.tile as tile
from concourse import bacc
from concourse.bass_utils import run_bass_kernel_spmd
from concourse.masks import make_identity

bf16 = ml_dtypes.bfloat16
f8e4 = ml_dtypes.float8_e4m3
FP32 = mybir.dt.float32
BF16 = mybir.dt.bfloat16
F8 = mybir.dt.float8e4
FP16 = mybir.dt.float16
U8 = mybir.dt.uint8
ALU = mybir.AluOpType
ACTF = mybir.ActivationFunctionType
DR = mybir.MatmulPerfMode.DoubleRow

# problem dims (hardcoded per contract)
NCORES = 8
B, PB, L, D, H, DT, T = 32, 256, 16, 832, 384, 768, 512
P = B * PB                      # 8192 pairs
PS = P // NCORES                # 1024 pairs per core
NB = B // NCORES                # 4 batches per core
C = 512                         # pair-chunk (matmul moving free dim)
NCH = PS // C                   # 2 chunks per core
H4 = 4 * H                      # 1536 gate features, column order i|o|f|u
M12 = H4 // 128                 # 12 m-tiles of gate features
NP6 = M12 // 2                  # 6 m-pairs (one [128,1024] PSUM pair each)
KH = H // 128                   # 3 k-subtiles of hidden
NKB = 5                         # DR blocks: 3x pure-x, x-tail|U0, U1|U2
WSC = 32.0                      # fp8 weight pre-scale
DEC_IN, DEC_H, DEC_OUT = 3 * H + 2 * DT, 512, 7
K21 = DEC_IN // 128             # 21 feature k-tiles for W1
M4 = DEC_H // 128               # 4 m-tiles for W1 output
MT = DT // 128                  # 6 span-feature m-tiles
JT = PB // 128                  # 2 pair-tiles per batch (for masks)


def _build_program(debug: bool = False, loop_n: int = 0,
                   widths=None) -> bass.Bass:
    """loop_n > 0 wraps the whole body in a For_i loop executing it loop_n
    times (identical work each iteration) — used only for timing via
    (T(N) - T(1)) / (N - 1).

    widths[s] = active pair-column count per 256-block at up-dir step s
    (pairs are host-sorted by root_idx within each batch, so up-direction
    matmuls only touch the suffix window that still has unreached roots).
    None -> full width. Defaults to the widths stashed by _prep_in_maps."""
    if widths is None:
        widths = _CACHE.get("widths", (256,) * L)
    nc = bacc.Bacc("TRN2", target_bir_lowering=False, debug=False,
                   num_devices=NCORES)
    dp = nc.declare_dram_parameter
    if debug:
        dbg_span = dp("dbg_span", [2, MT, 128, PS], BF16, isOutput=True)
        dbg_racc = dp("dbg_racc", [NCH, KH, 128, C], BF16, isOutput=True)
        dbg_start = dp("dbg_start", [NCH, 128, KH, C], BF16, isOutput=True)
        dbg_end = dp("dbg_end", [NCH, 128, KH, C], BF16, isOutput=True)

    node_dr = dp("node_dr", [L, 128, 7, PS], F8, isOutput=False)
    tok = dp("tok", [NB, T, DT], BF16, isOutput=False)
    spb = dp("spb", [2, NB, 3 * PB], FP16, isOutput=False)
    mask8_d = dp("mask8_d", [L, 128, PS], U8, isOutput=False)
    Wu = dp("Wu", [128, NKB, 2, H4], F8, isOutput=False)
    Wd = dp("Wd", [128, NKB, 2, H4], F8, isOutput=False)
    W1 = dp("W1", [DEC_IN, DEC_H], BF16, isOutput=False)
    W2 = dp("W2", [DEC_H, DEC_OUT], BF16, isOutput=False)
    b1 = dp("b1", [M4, 128, 1], FP32, isOutput=False)
    b2 = dp("b2", [DEC_OUT, 1], FP32, isOutput=False)
    ones = dp("ones", [1, 128], FP16, isOutput=False)
    iota_c = dp("iota_c", [128, T // 128], FP32, isOutput=False)
    out_d = dp("out", [DEC_OUT, PS], FP32, isOutput=True)

    def loadc(pool, name, src_ap, shape, dtype, bufs=1):
        t = pool.tile(shape, dtype, name=name, tag=name, bufs=bufs)
        nc.sync.dma_start(t[:], src_ap)
        return t

    with tile.TileContext(nc) as tc, ExitStack() as ctx:
        if loop_n:
            ctx.enter_context(tc.For_i(0, loop_n, 1))
        # whole-program pools
        cpool = ctx.enter_context(tc.tile_pool(name="const", bufs=1))
        spanp = ctx.enter_context(tc.tile_pool(name="spanp", bufs=1))
        capp = ctx.enter_context(tc.tile_pool(name="capp", bufs=1))

        # spanT[sp][m]: [128, PS] bf16 feature-major span means (whole program)
        spanT = [[spanp.tile([128, PS], BF16, name=f"span{sp}_{m}",
                             tag=f"span{sp}_{m}") for m in range(MT)]
                 for sp in range(2)]
        # span-feature partial of the MLP first layer, computed in phase 1
        # (PSUM free there) and re-added in phase 3: [128, C] bf16 per (m, ch)
        z_span = [[spanp.tile([128, C], BF16, name=f"zs{m}_{ch}",
                              tag=f"zs{m}_{ch}") for ch in range(NCH)]
                  for m in range(M4)]
        # per-chunk LSTM summary tiles (whole program; consumed by the MLP)
        root_acc = [[capp.tile([128, C], BF16, name=f"racc{ch}_{k}",
                               tag=f"racc{ch}_{k}") for k in range(KH)]
                    for ch in range(NCH)]
        start_t = [None] * NCH
        end_t = [None] * NCH

        # ---- phase 1: span mean pooling --------------------------------
        # span masks are built directly in transposed [token(part), pair]
        # layout: st/en/recip are PE-broadcast across partitions (fp16 ones
        # outer product), then compared against a per-partition token iota.
        # The recip scale is applied after the pooling matmul.
        with tc.tile_pool(name="tokp", bufs=2) as tokp, \
             tc.tile_pool(name="mwork", bufs=2) as mwork, \
             tc.tile_pool(name="spsum", bufs=4, space="PSUM") as spsum:
            iota_ct = loadc(tokp, "iotac", iota_c[:, :], [128, T // 128], FP32)

            def load_tok(b):
                tk = []
                for tb in range(T // 128):
                    t = tokp.tile([128, DT], BF16, name=f"tok{tb}",
                                  tag=f"tok{tb}")
                    nc.sync.dma_start(t[:], tok[b, tb * 128:(tb + 1) * 128, :])
                    tk.append(t)
                return tk

            tk0 = load_tok(0)
            w1s_t = [loadc(mwork, f"w1s{k}", W1[(9 + k) * 128:(10 + k) * 128, :],
                           [128, DEC_H], BF16) for k in range(2 * MT)]
            # heavy constant DMAs on the gpsimd DGE queue: they run in
            # parallel with the phase-1-critical SP-queue loads above
            b1_t = [loadc(cpool, f"b1{m}", b1[m], [128, 1], FP32)
                    for m in range(M4)]
            b2_t = loadc(cpool, "b2t", b2[:, :], [DEC_OUT, 1], FP32)
            ones_t = cpool.tile([1, 128], FP16, name="onest", tag="onest")
            nc.gpsimd.dma_start(ones_t[:], ones[:, :])
            w_t = {}
            for d, W in (("u", Wu), ("d", Wd)):
                w_t[d] = cpool.tile([128, NKB, 2, H4], F8, name=f"wdr{d}",
                                    tag=f"wdr{d}")
                nc.gpsimd.dma_start(w_t[d][:], W[:, :, :, :])
            mask8 = []
            for s in range(L):
                m8 = capp.tile([128, PS], U8, name=f"mask{s}", tag=f"mask{s}")
                nc.gpsimd.dma_start(m8[:], mask8_d[s])
                mask8.append(m8)
            for b in range(NB):
                tk = tk0 if b == 0 else load_tok(b)
                bcs_sp = {}
                maskT_sp = {}
                for sp in range(2):
                    spt = mwork.tile([1, 3 * PB], FP16, name="spt", tag="spt",
                                     bufs=4)
                    nc.sync.dma_start(spt[:], spb[sp, b])
                    bc_ps = spsum.tile([128, 3 * PB], FP32, name="bc",
                                       tag="bc", bufs=2)
                    for r in range(3):
                        nc.tensor.matmul(bc_ps[:, r * PB:(r + 1) * PB],
                                         ones_t[:],
                                         spt[:, r * PB:(r + 1) * PB],
                                         start=True, stop=True)
                    bcs = mwork.tile([128, 3, PB], FP32, name="bcs", tag="bcs")
                    nc.vector.tensor_copy(bcs[:], bc_ps[:])
                    maskT = [mwork.tile([128, PB], BF16, name=f"mT{tb}",
                                        tag=f"mT{tb}") for tb in range(T // 128)]
                    for tb in range(T // 128):
                        c1 = mwork.tile([128, PB], BF16, name="c1", tag="c1",
                                        bufs=4)
                        c2 = mwork.tile([128, PB], BF16, name="c2", tag="c2",
                                        bufs=4)
                        nc.vector.tensor_scalar(c1[:], bcs[:, 0, :],
                                                iota_ct[:, tb:tb + 1], None,
                                                ALU.is_le)
                        nc.vector.tensor_scalar(c2[:], bcs[:, 1, :],
                                                iota_ct[:, tb:tb + 1], None,
                                                ALU.is_gt)
                        nc.gpsimd.tensor_tensor(maskT[tb][:], c1[:], c2[:],
                                                ALU.mult)
                    bcs_sp[sp] = bcs
                    maskT_sp[sp] = maskT
                # both span groups' pooling matmuls interleaved per (m, tb)
                # so each stationary token tile is loaded once, not twice
                for m in range(MT):
                    zp_sp = {sp: spsum.tile([128, PB], FP32, name="zp",
                                            tag="mm") for sp in range(2)}
                    for tb in range(T // 128):
                        for sp in range(2):
                            nc.tensor.matmul(zp_sp[sp][:],
                                             tk[tb][:, m * 128:(m + 1) * 128],
                                             maskT_sp[sp][tb][:],
                                             start=(tb == 0),
                                             stop=(tb == T // 128 - 1))
                    for sp in range(2):
                        nc.vector.tensor_tensor(
                            spanT[sp][m][:, b * PB:(b + 1) * PB], zp_sp[sp][:],
                            bcs_sp[sp][:, 2, :], ALU.mult)
                # span-feature partial z for chunk ch = b//2: 12 of the 21 W1
                # k-tiles contract here, filling PE slack off the phase-3 tail
                if b % 2 == 1:
                    ch = b // 2
                    c0 = ch * C
                    sfeats = ([spanT[0][m][:, c0:c0 + C] for m in range(MT)]
                              + [spanT[1][m][:, c0:c0 + C] for m in range(MT)])
                    for m in range(M4):
                        zps = spsum.tile([128, C], FP32, name="zps", tag="mm")
                        for k in range(2 * MT):
                            nc.tensor.matmul(zps[:],
                                             w1s_t[k][:, m * 128:(m + 1) * 128],
                                             sfeats[k], start=(k == 0),
                                             stop=(k == 2 * MT - 1))
                        nc.vector.tensor_copy(z_span[m][ch][:], zps[:])

        # ---- phase 2: bidirectional chain-LSTM, fp8 DoubleRow ----------
        with tc.tile_pool(name="nodep", bufs=2) as nodep, \
             tc.tile_pool(name="cstp", bufs=2) as cstp, \
             tc.tile_pool(name="hdrp", bufs=2) as hdrp, \
             tc.tile_pool(name="gatep", bufs=8) as gatep, \
             tc.tile_pool(name="scrp", bufs=2) as scrp, \
             tc.tile_pool(name="pmm", bufs=4, space="PSUM") as pmm:
            for ch in range(NCH):
                for k in range(KH):
                    nc.vector.memset(root_acc[ch][k][:], 0.0)

            def new_node_tile(d, ch, t_src, memset7):
                t = nodep.tile([128, 8, C], F8, name=f"nd_{d}{ch}",
                               tag=f"nd_{d}{ch}")
                nc.sync.dma_start(t[:, 0:7, :],
                                  node_dr[t_src, :, :, ch * C:(ch + 1) * C])
                if memset7:
                    nc.vector.memset(t[:, 7:8, :], 0.0)
                return t

            nd_cur = {}
            for d in ("u", "d"):
                for ch in range(NCH):
                    nd_cur[d, ch] = new_node_tile(
                        d, ch, 0 if d == "u" else L - 1, True)
            cst = {}
            hdr = {}

            for s in range(L):
                for d in ("u", "d"):
                    nd_nx = {}
                    if s + 1 < L:
                        t_src = (s + 1) if d == "u" else L - 2 - s
                        for ch in range(NCH):
                            nd_nx[ch] = new_node_tile(d, ch, t_src, False)

                    # -- gate matmuls: 6 m-pairs, software-pipelined so the
                    # h-dependent blocks (kb3/kb4) of pair j are emitted after
                    # the x-only blocks (kb0..2) of pair j+1.
                    pm = {}

                    # at s==0 the f-gates are unused (c_prev == 0): skip m
                    # pair 0 (f0,f1) entirely.
                    if s == 0:
                        halves = {0: (), 1: (0, 1), 2: (0, 1), 3: (0, 1),
                                  4: (0, 1), 5: (0, 1)}
                    else:
                        halves = {j: (0, 1) for j in range(NP6)}

                    # up-dir active window: pairs sorted by root within
                    # each 256-block; only the suffix [o, 256) of each block
                    # still needs the recurrence at step s. Every engine
                    # (PE, ACT, DVE, Pool, captures) touches exactly these
                    # segments, so no tile ever carries a partially-stale
                    # region that a later full-width op would read.
                    if d == "u" and widths[s] <= 248:
                        o = 256 - widths[s]
                        op_segs = [(o, widths[s]), (256 + o, widths[s])]
                    else:
                        op_segs = [(0, 512)]
                    mm_segs = op_segs
                    nsg_mm = len(mm_segs)

                    def emit_x(j):
                        if not halves[j]:
                            return
                        for ch in range(NCH):
                            pm[j, ch] = pmm.tile([128, 2, C], FP32, name="pm",
                                                 tag="mm")
                        for half in halves[j]:
                            m = 2 * j + half
                            for kb in range(3):
                                for ch in range(NCH):
                                    for sg, (off, w) in enumerate(mm_segs):
                                        nc.tensor.matmul(
                                            pm[j, ch][:, half, off:off + w],
                                            w_t[d][:, kb, :,
                                                   m * 128:(m + 1) * 128],
                                            nd_cur[d, ch][:, 2 * kb:2 * kb + 2,
                                                          off:off + w],
                                            start=(kb == 0 and sg == 0),
                                            stop=False, perf_mode=DR)

                    def emit_h(j):
                        for half in halves[j]:
                            m = 2 * j + half
                            for ch in range(NCH):
                                for sg, (off, w) in enumerate(mm_segs):
                                    nc.tensor.matmul(
                                        pm[j, ch][:, half, off:off + w],
                                        w_t[d][:, 3, :, m * 128:(m + 1) * 128],
                                        nd_cur[d, ch][:, 6:8, off:off + w],
                                        start=False,
                                        stop=(s == 0 and sg == nsg_mm - 1),
                                        perf_mode=DR)
                            if s > 0:
                                for ch in range(NCH):
                                    for sg, (off, w) in enumerate(mm_segs):
                                        nc.tensor.matmul(
                                            pm[j, ch][:, half, off:off + w],
                                            w_t[d][:, 4, :,
                                                   m * 128:(m + 1) * 128],
                                            hdr[d, ch][:, :, off:off + w],
                                            start=False,
                                            stop=(sg == nsg_mm - 1),
                                            perf_mode=DR)

                    gates = {ch: {} for ch in range(NCH)}

                    def emit_act(j):
                        # gate column order f|u|i|o: pairs 0,3,4,5 sigmoid,
                        # pair 1 = (f2|u0) mixed, pair 2 = (u1,u2) tanh.
                        # Reads cover exactly the matmul-written segments.
                        if not halves[j]:
                            for ch in range(NCH):
                                gates[ch][j] = None
                            return
                        for ch in range(NCH):
                            g = gatep.tile([128, 2, C], BF16, name="g", tag="g")
                            src = pm[j, ch]
                            if j == 1:
                                for off, w in op_segs:
                                    nc.scalar.activation(g[:, 0, off:off + w],
                                                         src[:, 0, off:off + w],
                                                         ACTF.Sigmoid,
                                                         scale=1.0 / WSC)
                                    nc.scalar.activation(g[:, 1, off:off + w],
                                                         src[:, 1, off:off + w],
                                                         ACTF.Tanh,
                                                         scale=1.0 / WSC)
                            else:
                                fn = ACTF.Tanh if j == 2 else ACTF.Sigmoid
                                for off, w in op_segs:
                                    nc.scalar.activation(g[:, :, off:off + w],
                                                         src[:, :, off:off + w],
                                                         fn, scale=1.0 / WSC)
                            gates[ch][j] = g

                    emit_x(0)
                    for j in range(NP6):
                        if j + 1 < NP6:
                            emit_x(j + 1)
                        emit_h(j)
                        emit_act(j)

                    # -- state update per chunk, window-segmented. All gate
                    # activations are queued in j-order above, so the PSUM
                    # pool recycles at gate-ACT cadence and the next
                    # step-dir's x-matmuls never wait on this (DVE/ACT) tail.
                    hb0 = {}
                    hb12 = {}
                    tcs = {}
                    for ch in range(NCH):
                        gs = gates[ch]
                        u_ = [gs[1][:, 1, :], gs[2][:, 0, :], gs[2][:, 1, :]]
                        i_ = [gs[3][:, 0, :], gs[3][:, 1, :], gs[4][:, 0, :]]
                        cn = cstp.tile([128, KH, C], BF16, name=f"c_{d}{ch}",
                                       tag=f"c_{d}{ch}")
                        if s == 0:
                            for k in range(KH):
                                for off, w in op_segs:
                                    nc.vector.tensor_tensor(
                                        cn[:, k, off:off + w],
                                        i_[k][:, off:off + w],
                                        u_[k][:, off:off + w], ALU.mult)
                        else:
                            f2 = gs[1][:, 0, :]
                            tmp = scrp.tile([128, KH, C], BF16, name="tmp",
                                            tag="tmp", bufs=2)
                            for k in range(KH):
                                for off, w in op_segs:
                                    nc.vector.tensor_tensor(
                                        tmp[:, k, off:off + w],
                                        i_[k][:, off:off + w],
                                        u_[k][:, off:off + w], ALU.mult)
                            for off, w in op_segs:
                                nc.vector.tensor_tensor(
                                    cn[:, 0:2, off:off + w],
                                    gs[0][:, :, off:off + w],
                                    cst[d, ch][:, 0:2, off:off + w], ALU.mult)
                                nc.vector.tensor_tensor(
                                    cn[:, 2, off:off + w],
                                    f2[:, off:off + w],
                                    cst[d, ch][:, 2, off:off + w], ALU.mult)
                                nc.vector.tensor_tensor(
                                    cn[:, :, off:off + w],
                                    cn[:, :, off:off + w],
                                    tmp[:, :, off:off + w], ALU.add)
                        cst[d, ch] = cn
                        tc_ = scrp.tile([128, KH, C], BF16, name="tc", tag="tc",
                                        bufs=3)
                        for off, w in op_segs:
                            nc.scalar.activation(tc_[:, :, off:off + w],
                                                 cn[:, :, off:off + w],
                                                 ACTF.Tanh)
                        tcs[ch] = tc_
                        h0 = scrp.tile([128, C], BF16, name="hb0", tag="hb0",
                                       bufs=3)
                        for off, w in op_segs:
                            nc.vector.tensor_tensor(h0[:, off:off + w],
                                                    gs[4][:, 1, off:off + w],
                                                    tc_[:, 0, off:off + w],
                                                    ALU.mult)
                        hb0[ch] = h0
                        if s + 1 < L:
                            for off, w in op_segs:
                                nc.vector.tensor_copy(
                                    nd_nx[ch][:, 7:8, off:off + w],
                                    h0[:, off:off + w])
                    for ch in range(NCH):
                        gs = gates[ch]
                        h12 = scrp.tile([128, 2, C], BF16, name="hb12",
                                        tag="hb12", bufs=3)
                        for off, w in op_segs:
                            nc.vector.tensor_tensor(
                                h12[:, :, off:off + w],
                                gs[5][:, :, off:off + w],
                                tcs[ch][:, 1:KH, off:off + w], ALU.mult)
                        hb12[ch] = h12
                        if s + 1 < L:
                            hd = hdrp.tile([128, 2, C], F8, name=f"h_{d}{ch}",
                                           tag=f"h_{d}{ch}")
                            for off, w in op_segs:
                                nc.gpsimd.tensor_copy(hd[:, :, off:off + w],
                                                      h12[:, :, off:off + w])
                            hdr[d, ch] = hd
                    for ch in range(NCH):
                        if d == "u":
                            for off, w in op_segs:
                                mseg = mask8[s][:, ch * C + off:
                                                ch * C + off + w]
                                nc.vector.copy_predicated(
                                    root_acc[ch][0][:, off:off + w], mseg,
                                    hb0[ch][:, off:off + w])
                                for k in range(1, KH):
                                    nc.vector.copy_predicated(
                                        root_acc[ch][k][:, off:off + w], mseg,
                                        hb12[ch][:, k - 1, off:off + w])
                        else:
                            if s == 0:
                                end_t[ch] = capp.tile([128, KH, C], BF16,
                                                      name=f"end{ch}",
                                                      tag=f"end{ch}")
                                nc.gpsimd.tensor_copy(end_t[ch][:, 0, :],
                                                      hb0[ch][:])
                                nc.gpsimd.tensor_copy(end_t[ch][:, 1:KH, :],
                                                      hb12[ch][:])
                            if s == L - 1:
                                start_t[ch] = capp.tile([128, KH, C], BF16,
                                                        name=f"start{ch}",
                                                        tag=f"start{ch}")
                                nc.gpsimd.tensor_copy(start_t[ch][:, 0, :],
                                                      hb0[ch][:])
                                nc.gpsimd.tensor_copy(start_t[ch][:, 1:KH, :],
                                                      hb12[ch][:])
                    if s + 1 < L:
                        for ch in range(NCH):
                            nd_cur[d, ch] = nd_nx[ch]

        if debug:
            for sp in range(2):
                for m in range(MT):
                    nc.sync.dma_start(dbg_span[sp, m], spanT[sp][m][:])
            for ch in range(NCH):
                for k in range(KH):
                    nc.sync.dma_start(dbg_racc[ch, k], root_acc[ch][k][:])
                nc.sync.dma_start(dbg_start[ch], start_t[ch][:])
                nc.sync.dma_start(dbg_end[ch], end_t[ch][:])

        # ---- phase 3: pair MLP -----------------------------------------
        with tc.tile_pool(name="mlpw", bufs=1) as mlpw, \
             tc.tile_pool(name="mlpp", bufs=4) as mlpp, \
             tc.tile_pool(name="mpsum", bufs=6, space="PSUM") as mpsum, \
             tc.tile_pool(name="pout", bufs=1, space="PSUM") as pout:
            w1_t = [loadc(mlpw, f"w1{k}", W1[k * 128:(k + 1) * 128, :],
                          [128, DEC_H], BF16) for k in range(3 * KH)]
            w2_t = [loadc(mlpw, f"w2{k}", W2[k * 128:(k + 1) * 128, :],
                          [128, DEC_OUT], BF16) for k in range(M4)]
            for ch in range(NCH):
                c0 = ch * C
                feats = ([root_acc[ch][k][:] for k in range(KH)]
                         + [start_t[ch][:, k, :] for k in range(KH)]
                         + [end_t[ch][:, k, :] for k in range(KH)])
                z_t = []
                for m in range(M4):
                    zp = mpsum.tile([128, C], FP32, name="zp2", tag="mm")
                    for k in range(3 * KH):
                        nc.tensor.matmul(zp[:], w1_t[k][:, m * 128:(m + 1) * 128],
                                         feats[k], start=(k == 0),
                                         stop=(k == 3 * KH - 1))
                    zs = mlpp.tile([128, C], BF16, name="zs", tag="zsum")
                    nc.vector.tensor_tensor(zs[:], zp[:], z_span[m][ch][:],
                                            ALU.add)
                    z = mlpp.tile([128, C], BF16, name="z", tag="z")
                    nc.scalar.activation(z[:], zs[:], ACTF.Tanh, bias=b1_t[m][:])
                    z_t.append(z)
                op = pout.tile([DEC_OUT, C], FP32, name="op", tag="op")
                for m in range(M4):
                    nc.tensor.matmul(op[:], w2_t[m][:], z_t[m][:], start=(m == 0),
                                     stop=(m == M4 - 1))
                osb = mlpp.tile([DEC_OUT, C], FP32, name="osb", tag="osb", bufs=2)
                nc.vector.tensor_scalar(osb[:], op[:], b2_t[:], None, ALU.add)
                nc.sync.dma_start(out_d[:, c0:c0 + C], osb[:])

    nc.compile()
    _dedupe_ldweights(nc)
    return nc


def _dedupe_ldweights(nc):
    """Remove PE InstLdweights whose weights AP equals the most recently
    retained one with only PE Matmults in between (the PE weight buffer is
    unchanged by other engines). Only wait-free/update-free loads are removed."""
    import concourse.mybir as _mb
    for name, bb in list(nc.bb_map.items()):
        insts = bb.bb.instructions
        out = []
        prev_sig = None
        removed = 0
        for inst in insts:
            tn = type(inst).__name__
            eng = getattr(inst, "engine", None)
            if eng == _mb.EngineType.PE:
                if tn == "InstLdweights":
                    si = inst.sync_info
                    clean = si is None or (not si.on_wait and not si.on_update)
                    try:
                        sig = str(inst.ins[0])
                    except Exception:
                        sig = None
                    if clean and sig is not None and sig == prev_sig:
                        removed += 1
                        continue
                    prev_sig = sig
                elif tn != "InstMatmult":
                    prev_sig = None
            out.append(inst)
        if removed:
            bb.bb.instructions = out


_CACHE = {}


def _get_program() -> bass.Bass:
    if "nc" not in _CACHE:
        _CACHE["nc"] = _build_program()
    return _CACHE["nc"]


def _prep_in_maps(inputs) -> list[dict]:
    f32 = np.float32
    node = np.asarray(inputs["node_embs"], f32)
    tokf = np.asarray(inputs["token_embs"], f32)
    rooti = np.asarray(inputs["root_idx"])

    # sort pairs by root_idx within each batch (pure layout permutation,
    # inverted on the output): enables the up-dir early-exit windows
    rb = rooti.reshape(B, PB)
    order = np.argsort(rb, axis=1, kind="stable")           # [B, PB]
    perm_full = (order + (np.arange(B) * PB)[:, None]).reshape(P)
    _CACHE["inv_perm"] = np.argsort(perm_full)
    node = node[perm_full]
    rooti = rooti[perm_full]
    rb_s = rooti.reshape(B, PB)
    # widths[s] = 256 - min over batches of #(root < s); exact cover for
    # every batch block on every core
    cnt = (rb_s[:, :, None] < np.arange(L)[None, None, :]).sum(1)   # [B, L]
    _CACHE["widths"] = tuple(int(256 - c) for c in cnt.min(0))

    def bperm(a):
        return np.take_along_axis(np.asarray(a), order, axis=1)

    inputs = dict(inputs,
                  p1_st=bperm(inputs["p1_st"]), p1_len=bperm(inputs["p1_len"]),
                  p2_st=bperm(inputs["p2_st"]), p2_len=bperm(inputs["p2_len"]))
    # [P, L, D] fp32 -> per-core [L, 128, 7, PS] fp8 with bias row appended:
    # rows 0..831 = x, row 832 = 1.0, rows 833..895 = 0 (7 k-subtiles of 128)
    n8 = node.astype(f8e4).reshape(NCORES, PS, L, D).transpose(0, 2, 3, 1)
    pad = np.zeros((NCORES, L, 896, PS), f8e4)
    pad[:, :, :D, :] = n8
    pad[:, :, D, :] = f8e4(1.0)
    node_sh = np.ascontiguousarray(
        pad.reshape(NCORES, L, 7, 128, PS).transpose(0, 1, 3, 2, 4))
    tok_sh = tokf.reshape(NCORES, NB, T, DT).astype(bf16)

    def span_arrays(st, ln):
        st = np.asarray(st).astype(f32)
        ln = np.asarray(ln).astype(f32)
        en = st + ln + 1.0
        rc = 1.0 / (ln + 1.0)
        return st, en, rc

    s1, e1, r1 = span_arrays(inputs["p1_st"], inputs["p1_len"])
    s2, e2, r2 = span_arrays(inputs["p2_st"], inputs["p2_len"])

    def pack_span(a1, a2):
        # [B, PB] x2 -> per-core [2, NB, PB]
        a = np.stack([a1, a2])  # [2, B, PB]
        return a.reshape(2, NCORES, NB, PB).transpose(1, 0, 2, 3)

    # [NCORES, 2, NB, 3, PB] fp16 rows: st | en | recip (values <= 503 are
    # exact in fp16; recip has 10-bit mantissa)
    spb = np.ascontiguousarray(np.stack(
        [pack_span(s1, s2), pack_span(e1, e2), pack_span(r1, r2)],
        axis=3).astype(np.float16)).reshape(NCORES, 2, NB, 3 * PB)

    # one-hot root masks, broadcast across partitions: [NCORES, L, 128, PS]
    oh = (rooti.reshape(NCORES, 1, PS) ==
          np.arange(L, dtype=rooti.dtype).reshape(1, L, 1))
    mask8_h = np.ascontiguousarray(np.broadcast_to(
        oh[:, :, None, :], (NCORES, L, 128, PS)).astype(np.uint8))

    # gate-column permutation i|o|u|f -> f|u|i|o
    perm = np.concatenate([np.arange(3 * H, 4 * H), np.arange(2 * H, 3 * H),
                           np.arange(0, H), np.arange(H, 2 * H)])

    def build_wdr(Wiou, Wf, Uiou, Uf, biou, bf):
        W = np.concatenate([np.asarray(Wiou, f32), np.asarray(Wf, f32)],
                           axis=1)[:, perm]      # [D, 4H]
        U = np.concatenate([np.asarray(Uiou, f32), np.asarray(Uf, f32)],
                           axis=1)[:, perm]      # [H, 4H]
        b = np.concatenate([np.asarray(biou, f32), np.asarray(bf, f32)])[perm]
        Wpad = np.zeros((NKB * 256, H4), f32)
        Wpad[:D] = W
        Wpad[D] = b
        Wpad[896:896 + H] = U
        W8 = (Wpad * WSC).astype(f8e4)
        # [5 kb, 2 grp, 128 p, H4] -> [128, 5, 2, H4]
        return np.ascontiguousarray(
            W8.reshape(NKB, 2, 128, H4).transpose(2, 0, 1, 3))

    Wu_h = build_wdr(inputs["Wiou_u"], inputs["Wf_u"], inputs["Uiou_u"],
                     inputs["Uf_u"], inputs["biou_u"], inputs["bf_u"])
    Wd_h = build_wdr(inputs["Wiou_d"], inputs["Wf_d"], inputs["Uiou_d"],
                     inputs["Uf_d"], inputs["biou_d"], inputs["bf_d"])
    W1_h = np.asarray(inputs["W1"], f32).astype(bf16)
    W2_h = np.asarray(inputs["W2"], f32).astype(bf16)
    b1_h = np.asarray(inputs["b1"], f32).reshape(M4, 128, 1)
    b2_h = np.asarray(inputs["b2"], f32).reshape(DEC_OUT, 1)
    ones_h = np.ones((1, 128), np.float16)
    iota_h = np.ascontiguousarray(
        (np.arange(T // 128, dtype=f32)[None, :] * 128
         + np.arange(128, dtype=f32)[:, None]))

    in_maps = []
    for c in range(NCORES):
        in_maps.append({
            "node_dr": node_sh[c], "tok": tok_sh[c],
            "spb": spb[c], "mask8_d": mask8_h[c],
            "Wu": Wu_h, "Wd": Wd_h,
            "W1": W1_h, "W2": W2_h,
            "b1": b1_h, "b2": b2_h, "ones": ones_h, "iota_c": iota_h,
        })
    return in_maps


def unpermute(out_full: np.ndarray) -> np.ndarray:
    """Invert the host-side pair sort on a full [P, ...] output."""
    return out_full[_CACHE["inv_perm"]]


def run(inputs, **kwargs):
    """Run on hardware; returns (output [P, DEC_OUT] fp32, BassKernelResults)."""
    in_maps = _prep_in_maps(inputs)
    nc = _get_program()
    res = run_bass_kernel_spmd(nc, in_maps, list(range(NCORES)), **kwargs)
    outs = [np.asarray(r["out"], np.float32).T for r in res.results]  # [PS, 7]
    return unpermute(np.concatenate(outs, axis=0)), res


def kernel(**inputs) -> np.ndarray:
    out, _ = run(inputs)
    return out



# revision 14
# speedup vs baseline: 1.1785x; 1.1139x over previous
"""Trainium2 Bass kernel for nn_DepPairingLayer (bidirectional chain-TreeLSTM over
shortest-path node chains + span mean-pooling + pair MLP), SPMD across 8 NeuronCores.

Sharding: data-parallel over the pair dimension P=8192 (1024 pairs/core); all
weights replicated.

The LSTM matmuls (both the x-projection and the h-recurrence) run in fp8e4m3
with DoubleRow perf mode: each matmul contracts 256 rows (two 128-row groups
packed as [128, 2, free] tiles), halving PE streaming time vs bf16. The full
contraction per gate tile is 5 DR blocks covering exactly
[x(832) | bias-row(1) | pad | U(384)] = 1280 rows: the x-tail block pairs the
last 64 x-rows + the folded bias row with h-block0 (copied into the node tile's
8th slot each step), and the last block pairs h-block1/h-block2. Weights are
pre-scaled x32 for fp8 range; the gate activation applies scale 1/32.
Gate activations read [128,2,512] 2-bank PSUM pairs in wide ACT instructions
(gate column order re-packed to f|u|i|o so sigmoid/tanh runs are contiguous).
The root-selection masks are precomputed once.

Up-direction early exit: pairs are host-sorted by root_idx within each batch,
and once a 256-block's active suffix narrows (width <= 248) every engine --
PE matmuls, gate ACT, DVE state math, h copies, root captures -- operates on
exactly the two active segments, so no tile carries a partially-stale region.
At s==0 the f-gate m-pair is skipped entirely (c_prev == 0). All gate
activations are emitted in j-order so the PSUM ring recycles at ACT cadence.
The span-feature half of the MLP first layer (12 of 21 k-tiles) is contracted
during phase 1 per chunk as soon as its two batches finish pooling (PSUM and
PE slack exist there) and re-added via DVE in phase 3, shortening the serial
MLP tail. Everything else (span pooling, captures, MLP) stays bf16; measured
end-to-end rel-absmax error vs the fp32 reference: ~9e-3 (threshold 2e-2).
"""

from contextlib import ExitStack

import numpy as np
import ml_dtypes

import concourse.bass as bass
import concourse.mybir as mybir
import concourse.tile as tile
from concourse import bacc
from concourse.bass_utils import run_bass_kernel_spmd
from concourse.masks import make_identity

bf16 = ml_dtypes.bfloat16
f8e4 = ml_dtypes.float8_e4m3
FP32 = mybir.dt.float32
BF16 = mybir.dt.bfloat16
F8 = mybir.dt.float8e4
FP16 = mybir.dt.float16
U8 = mybir.dt.uint8
ALU = mybir.AluOpType
ACTF = mybir.ActivationFunctionType
DR = mybir.MatmulPerfMode.DoubleRow

# problem dims (hardcoded per contract)
NCORES = 8
B, PB, L, D, H, DT, T = 32, 256, 16, 832, 384, 768, 512
P = B * PB                      # 8192 pairs
PS = P // NCORES                # 1024 pairs per core
NB = B // NCORES                # 4 batches per core
C = 512                         # pair-chunk (matmul moving free dim)
NCH = PS // C                   # 2 chunks per core
H4 = 4 * H                      # 1536 gate features, column order i|o|f|u
M12 = H4 // 128                 # 12 m-tiles of gate features
NP6 = M12 // 2                  # 6 m-pairs (one [128,1024] PSUM pair each)
KH = H // 128                   # 3 k-subtiles of hidden
NKB = 5                         # DR blocks: 3x pure-x, x-tail|U0, U1|U2
WSC = 32.0                      # fp8 weight pre-scale
DEC_IN, DEC_H, DEC_OUT = 3 * H + 2 * DT, 512, 7
K21 = DEC_IN // 128             # 21 feature k-tiles for W1
M4 = DEC_H // 128               # 4 m-tiles for W1 output
MT = DT // 128                  # 6 span-feature m-tiles
JT = PB // 128                  # 2 pair-tiles per batch (for masks)


def _build_program(debug: bool = False, loop_n: int = 0,
                   widths=None) -> bass.Bass:
    """loop_n > 0 wraps the whole body in a For_i loop executing it loop_n
    times (identical work each iteration) — used only for timing via
    (T(N) - T(1)) / (N - 1).

    widths[s] = active pair-column count per 256-block at up-dir step s
    (pairs are host-sorted by root_idx within each batch, so up-direction
    matmuls only touch the suffix window that still has unreached roots).
    None -> full width. Defaults to the widths stashed by _prep_in_maps."""
    if widths is None:
        widths = _CACHE.get("widths", (256,) * L)
    nc = bacc.Bacc("TRN2", target_bir_lowering=False, debug=False,
                   num_devices=NCORES)
    dp = nc.declare_dram_parameter
    if debug:
        dbg_span = dp("dbg_span", [2, MT, 128, PS], BF16, isOutput=True)
        dbg_racc = dp("dbg_racc", [NCH, KH, 128, C], BF16, isOutput=True)
        dbg_start = dp("dbg_start", [NCH, 128, KH, C], BF16, isOutput=True)
        dbg_end = dp("dbg_end", [NCH, 128, KH, C], BF16, isOutput=True)

    node_dr = dp("node_dr", [L, 128, 7, PS], F8, isOutput=False)
    tok = dp("tok", [NB, T, DT], BF16, isOutput=False)
    spb = dp("spb", [2, NB, 3 * PB], FP16, isOutput=False)
    mask8_d = dp("mask8_d", [L, 128, PS], U8, isOutput=False)
    Wu = dp("Wu", [128, NKB, 2, H4], F8, isOutput=False)
    Wd = dp("Wd", [128, NKB, 2, H4], F8, isOutput=False)
    W1 = dp("W1", [DEC_IN, DEC_H], BF16, isOutput=False)
    W2 = dp("W2", [DEC_H, DEC_OUT], BF16, isOutput=False)
    b1 = dp("b1", [M4, 128, 1], FP32, isOutput=False)
    b2 = dp("b2", [DEC_OUT, 1], FP32, isOutput=False)
    ones = dp("ones", [1, 128], FP16, isOutput=False)
    iota_c = dp("iota_c", [128, T // 128], FP32, isOutput=False)
    out_d = dp("out", [DEC_OUT, PS], FP32, isOutput=True)

    def loadc(pool, name, src_ap, shape, dtype, bufs=1):
        t = pool.tile(shape, dtype, name=name, tag=name, bufs=bufs)
        nc.sync.dma_start(t[:], src_ap)
        return t

    with tile.TileContext(nc) as tc, ExitStack() as ctx:
        if loop_n:
            ctx.enter_context(tc.For_i(0, loop_n, 1))
        # whole-program pools
        cpool = ctx.enter_context(tc.tile_pool(name="const", bufs=1))
        spanp = ctx.enter_context(tc.tile_pool(name="spanp", bufs=1))
        capp = ctx.enter_context(tc.tile_pool(name="capp", bufs=1))

        # spanT[sp][m]: [128, PS] bf16 feature-major span means (whole program)
        spanT = [[spanp.tile([128, PS], BF16, name=f"span{sp}_{m}",
                             tag=f"span{sp}_{m}") for m in range(MT)]
                 for sp in range(2)]
        # span-feature partial of the MLP first layer, computed in phase 1
        # (PSUM free there) and re-added in phase 3: [128, C] bf16 per (m, ch)
        z_span = [[spanp.tile([128, C], BF16, name=f"zs{m}_{ch}",
                              tag=f"zs{m}_{ch}") for ch in range(NCH)]
                  for m in range(M4)]
        # per-chunk LSTM summary tiles (whole program; consumed by the MLP)
        root_acc = [[capp.tile([128, C], BF16, name=f"racc{ch}_{k}",
                               tag=f"racc{ch}_{k}") for k in range(KH)]
                    for ch in range(NCH)]
        start_t = [None] * NCH
        end_t = [None] * NCH

        # ---- phase 1: span mean pooling --------------------------------
        # span masks are built directly in transposed [token(part), pair]
        # layout: st/en/recip are PE-broadcast across partitions (fp16 ones
        # outer product), then compared against a per-partition token iota.
        # The recip scale is applied after the pooling matmul.
        with tc.tile_pool(name="tokp", bufs=2) as tokp, \
             tc.tile_pool(name="mwork", bufs=2) as mwork, \
             tc.tile_pool(name="spsum", bufs=4, space="PSUM") as spsum:
            iota_ct = loadc(tokp, "iotac", iota_c[:, :], [128, T // 128], FP32)

            def load_tok(b):
                tk = []
                for tb in range(T // 128):
                    t = tokp.tile([128, DT], BF16, name=f"tok{tb}",
                                  tag=f"tok{tb}")
                    nc.sync.dma_start(t[:], tok[b, tb * 128:(tb + 1) * 128, :])
                    tk.append(t)
                return tk

            tk0 = load_tok(0)
            w1s_t = [loadc(mwork, f"w1s{k}", W1[(9 + k) * 128:(10 + k) * 128, :],
                           [128, DEC_H], BF16) for k in range(2 * MT)]
            # heavy constant DMAs on the gpsimd DGE queue: they run in
            # parallel with the phase-1-critical SP-queue loads above
            b1_t = [loadc(cpool, f"b1{m}", b1[m], [128, 1], FP32)
                    for m in range(M4)]
            b2_t = loadc(cpool, "b2t", b2[:, :], [DEC_OUT, 1], FP32)
            ones_t = cpool.tile([1, 128], FP16, name="onest", tag="onest")
            nc.gpsimd.dma_start(ones_t[:], ones[:, :])
            w_t = {}
            for d, W in (("u", Wu), ("d", Wd)):
                w_t[d] = cpool.tile([128, NKB, 2, H4], F8, name=f"wdr{d}",
                                    tag=f"wdr{d}")
                nc.gpsimd.dma_start(w_t[d][:], W[:, :, :, :])
            mask8 = []
            for s in range(L):
                m8 = capp.tile([128, PS], U8, name=f"mask{s}", tag=f"mask{s}")
                nc.gpsimd.dma_start(m8[:], mask8_d[s])
                mask8.append(m8)
            for b in range(NB):
                tk = tk0 if b == 0 else load_tok(b)
                for sp in range(2):
                    spt = mwork.tile([1, 3 * PB], FP16, name="spt", tag="spt",
                                     bufs=4)
                    nc.sync.dma_start(spt[:], spb[sp, b])
                    bc_ps = spsum.tile([128, 3 * PB], FP32, name="bc",
                                       tag="bc", bufs=2)
                    for r in range(3):
                        nc.tensor.matmul(bc_ps[:, r * PB:(r + 1) * PB],
                                         ones_t[:],
                                         spt[:, r * PB:(r + 1) * PB],
                                         start=True, stop=True)
                    bcs = mwork.tile([128, 3, PB], FP32, name="bcs", tag="bcs")
                    nc.vector.tensor_copy(bcs[:], bc_ps[:])
                    maskT = [mwork.tile([128, PB], BF16, name=f"mT{tb}",
                                        tag=f"mT{tb}") for tb in range(T // 128)]
                    for tb in range(T // 128):
                        c1 = mwork.tile([128, PB], BF16, name="c1", tag="c1",
                                        bufs=4)
                        c2 = mwork.tile([128, PB], BF16, name="c2", tag="c2",
                                        bufs=4)
                        nc.vector.tensor_scalar(c1[:], bcs[:, 0, :],
                                                iota_ct[:, tb:tb + 1], None,
                                                ALU.is_le)
                        nc.vector.tensor_scalar(c2[:], bcs[:, 1, :],
                                                iota_ct[:, tb:tb + 1], None,
                                                ALU.is_gt)
                        nc.gpsimd.tensor_tensor(maskT[tb][:], c1[:], c2[:],
                                                ALU.mult)
                    for m in range(MT):
                        zp = spsum.tile([128, PB], FP32, name="zp", tag="mm")
                        for tb in range(T // 128):
                            nc.tensor.matmul(zp[:], tk[tb][:, m * 128:(m + 1) * 128],
                                             maskT[tb][:], start=(tb == 0),
                                             stop=(tb == T // 128 - 1))
                        nc.vector.tensor_tensor(
                            spanT[sp][m][:, b * PB:(b + 1) * PB], zp[:],
                            bcs[:, 2, :], ALU.mult)
                # span-feature partial z for chunk ch = b//2: 12 of the 21 W1
                # k-tiles contract here, filling PE slack off the phase-3 tail
                if b % 2 == 1:
                    ch = b // 2
                    c0 = ch * C
                    sfeats = ([spanT[0][m][:, c0:c0 + C] for m in range(MT)]
                              + [spanT[1][m][:, c0:c0 + C] for m in range(MT)])
                    for m in range(M4):
                        zps = spsum.tile([128, C], FP32, name="zps", tag="mm")
                        for k in range(2 * MT):
                            nc.tensor.matmul(zps[:],
                                             w1s_t[k][:, m * 128:(m + 1) * 128],
                                             sfeats[k], start=(k == 0),
                                             stop=(k == 2 * MT - 1))
                        nc.vector.tensor_copy(z_span[m][ch][:], zps[:])

        # ---- phase 2: bidirectional chain-LSTM, fp8 DoubleRow ----------
        with tc.tile_pool(name="nodep", bufs=2) as nodep, \
             tc.tile_pool(name="cstp", bufs=2) as cstp, \
             tc.tile_pool(name="hdrp", bufs=2) as hdrp, \
             tc.tile_pool(name="gatep", bufs=8) as gatep, \
             tc.tile_pool(name="scrp", bufs=2) as scrp, \
             tc.tile_pool(name="pmm", bufs=4, space="PSUM") as pmm:
            for ch in range(NCH):
                for k in range(KH):
                    nc.vector.memset(root_acc[ch][k][:], 0.0)

            def new_node_tile(d, ch, t_src, memset7):
                t = nodep.tile([128, 8, C], F8, name=f"nd_{d}{ch}",
                               tag=f"nd_{d}{ch}")
                nc.sync.dma_start(t[:, 0:7, :],
                                  node_dr[t_src, :, :, ch * C:(ch + 1) * C])
                if memset7:
                    nc.vector.memset(t[:, 7:8, :], 0.0)
                return t

            nd_cur = {}
            for d in ("u", "d"):
                for ch in range(NCH):
                    nd_cur[d, ch] = new_node_tile(
                        d, ch, 0 if d == "u" else L - 1, True)
            cst = {}
            hdr = {}

            for s in range(L):
                for d in ("u", "d"):
                    nd_nx = {}
                    if s + 1 < L:
                        t_src = (s + 1) if d == "u" else L - 2 - s
                        for ch in range(NCH):
                            nd_nx[ch] = new_node_tile(d, ch, t_src, False)

                    # -- gate matmuls: 6 m-pairs, software-pipelined so the
                    # h-dependent blocks (kb3/kb4) of pair j are emitted after
                    # the x-only blocks (kb0..2) of pair j+1.
                    pm = {}

                    # at s==0 the f-gates are unused (c_prev == 0): skip m
                    # pair 0 (f0,f1) entirely.
                    if s == 0:
                        halves = {0: (), 1: (0, 1), 2: (0, 1), 3: (0, 1),
                                  4: (0, 1), 5: (0, 1)}
                    else:
                        halves = {j: (0, 1) for j in range(NP6)}

                    # up-dir active window: pairs sorted by root within
                    # each 256-block; only the suffix [o, 256) of each block
                    # still needs the recurrence at step s. Every engine
                    # (PE, ACT, DVE, Pool, captures) touches exactly these
                    # segments, so no tile ever carries a partially-stale
                    # region that a later full-width op would read.
                    if d == "u" and widths[s] <= 248:
                        o = 256 - widths[s]
                        op_segs = [(o, widths[s]), (256 + o, widths[s])]
                    else:
                        op_segs = [(0, 512)]
                    mm_segs = op_segs
                    nsg_mm = len(mm_segs)

                    def emit_x(j):
                        if not halves[j]:
                            return
                        for ch in range(NCH):
                            pm[j, ch] = pmm.tile([128, 2, C], FP32, name="pm",
                                                 tag="mm")
                        for half in halves[j]:
                            m = 2 * j + half
                            for kb in range(3):
                                for ch in range(NCH):
                                    for sg, (off, w) in enumerate(mm_segs):
                                        nc.tensor.matmul(
                                            pm[j, ch][:, half, off:off + w],
                                            w_t[d][:, kb, :,
                                                   m * 128:(m + 1) * 128],
                                            nd_cur[d, ch][:, 2 * kb:2 * kb + 2,
                                                          off:off + w],
                                            start=(kb == 0 and sg == 0),
                                            stop=False, perf_mode=DR)

                    def emit_h(j):
                        for half in halves[j]:
                            m = 2 * j + half
                            for ch in range(NCH):
                                for sg, (off, w) in enumerate(mm_segs):
                                    nc.tensor.matmul(
                                        pm[j, ch][:, half, off:off + w],
                                        w_t[d][:, 3, :, m * 128:(m + 1) * 128],
                                        nd_cur[d, ch][:, 6:8, off:off + w],
                                        start=False,
                                        stop=(s == 0 and sg == nsg_mm - 1),
                                        perf_mode=DR)
                            if s > 0:
                                for ch in range(NCH):
                                    for sg, (off, w) in enumerate(mm_segs):
                                        nc.tensor.matmul(
                                            pm[j, ch][:, half, off:off + w],
                                            w_t[d][:, 4, :,
                                                   m * 128:(m + 1) * 128],
                                            hdr[d, ch][:, :, off:off + w],
                                            start=False,
                                            stop=(sg == nsg_mm - 1),
                                            perf_mode=DR)

                    gates = {ch: {} for ch in range(NCH)}

                    def emit_act(j):
                        # gate column order f|u|i|o: pairs 0,3,4,5 sigmoid,
                        # pair 1 = (f2|u0) mixed, pair 2 = (u1,u2) tanh.
                        # Reads cover exactly the matmul-written segments.
                        if not halves[j]:
                            for ch in range(NCH):
                                gates[ch][j] = None
                            return
                        for ch in range(NCH):
                            g = gatep.tile([128, 2, C], BF16, name="g", tag="g")
                            src = pm[j, ch]
                            if j == 1:
                                for off, w in op_segs:
                                    nc.scalar.activation(g[:, 0, off:off + w],
                                                         src[:, 0, off:off + w],
                                                         ACTF.Sigmoid,
                                                         scale=1.0 / WSC)
                                    nc.scalar.activation(g[:, 1, off:off + w],
                                                         src[:, 1, off:off + w],
                                                         ACTF.Tanh,
                                                         scale=1.0 / WSC)
                            else:
                                fn = ACTF.Tanh if j == 2 else ACTF.Sigmoid
                                for off, w in op_segs:
                                    nc.scalar.activation(g[:, :, off:off + w],
                                                         src[:, :, off:off + w],
                                                         fn, scale=1.0 / WSC)
                            gates[ch][j] = g

                    emit_x(0)
                    for j in range(NP6):
                        if j + 1 < NP6:
                            emit_x(j + 1)
                        emit_h(j)
                        emit_act(j)

                    # -- state update per chunk, window-segmented. All gate
                    # activations are queued in j-order above, so the PSUM
                    # pool recycles at gate-ACT cadence and the next
                    # step-dir's x-matmuls never wait on this (DVE/ACT) tail.
                    hb0 = {}
                    hb12 = {}
                    tcs = {}
                    for ch in range(NCH):
                        gs = gates[ch]
                        u_ = [gs[1][:, 1, :], gs[2][:, 0, :], gs[2][:, 1, :]]
                        i_ = [gs[3][:, 0, :], gs[3][:, 1, :], gs[4][:, 0, :]]
                        cn = cstp.tile([128, KH, C], BF16, name=f"c_{d}{ch}",
                                       tag=f"c_{d}{ch}")
                        if s == 0:
                            for k in range(KH):
                                for off, w in op_segs:
                                    nc.vector.tensor_tensor(
                                        cn[:, k, off:off + w],
                                        i_[k][:, off:off + w],
                                        u_[k][:, off:off + w], ALU.mult)
                        else:
                            f2 = gs[1][:, 0, :]
                            tmp = scrp.tile([128, KH, C], BF16, name="tmp",
                                            tag="tmp", bufs=2)
                            for k in range(KH):
                                for off, w in op_segs:
                                    nc.vector.tensor_tensor(
                                        tmp[:, k, off:off + w],
                                        i_[k][:, off:off + w],
                                        u_[k][:, off:off + w], ALU.mult)
                            for off, w in op_segs:
                                nc.vector.tensor_tensor(
                                    cn[:, 0:2, off:off + w],
                                    gs[0][:, :, off:off + w],
                                    cst[d, ch][:, 0:2, off:off + w], ALU.mult)
                                nc.vector.tensor_tensor(
                                    cn[:, 2, off:off + w],
                                    f2[:, off:off + w],
                                    cst[d, ch][:, 2, off:off + w], ALU.mult)
                                nc.vector.tensor_tensor(
                                    cn[:, :, off:off + w],
                                    cn[:, :, off:off + w],
                                    tmp[:, :, off:off + w], ALU.add)
                        cst[d, ch] = cn
                        tc_ = scrp.tile([128, KH, C], BF16, name="tc", tag="tc",
                                        bufs=3)
                        for off, w in op_segs:
                            nc.scalar.activation(tc_[:, :, off:off + w],
                                                 cn[:, :, off:off + w],
                                                 ACTF.Tanh)
                        tcs[ch] = tc_
                        h0 = scrp.tile([128, C], BF16, name="hb0", tag="hb0",
                                       bufs=3)
                        for off, w in op_segs:
                            nc.vector.tensor_tensor(h0[:, off:off + w],
                                                    gs[4][:, 1, off:off + w],
                                                    tc_[:, 0, off:off + w],
                                                    ALU.mult)
                        hb0[ch] = h0
                        if s + 1 < L:
                            for off, w in op_segs:
                                nc.vector.tensor_copy(
                                    nd_nx[ch][:, 7:8, off:off + w],
                                    h0[:, off:off + w])
                    for ch in range(NCH):
                        gs = gates[ch]
                        h12 = scrp.tile([128, 2, C], BF16, name="hb12",
                                        tag="hb12", bufs=3)
                        for off, w in op_segs:
                            nc.vector.tensor_tensor(
                                h12[:, :, off:off + w],
                                gs[5][:, :, off:off + w],
                                tcs[ch][:, 1:KH, off:off + w], ALU.mult)
                        hb12[ch] = h12
                        if s + 1 < L:
                            hd = hdrp.tile([128, 2, C], F8, name=f"h_{d}{ch}",
                                           tag=f"h_{d}{ch}")
                            for off, w in op_segs:
                                nc.gpsimd.tensor_copy(hd[:, :, off:off + w],
                                                      h12[:, :, off:off + w])
                            hdr[d, ch] = hd
                    for ch in range(NCH):
                        if d == "u":
                            for off, w in op_segs:
                                mseg = mask8[s][:, ch * C + off:
                                                ch * C + off + w]
                                nc.vector.copy_predicated(
                                    root_acc[ch][0][:, off:off + w], mseg,
                                    hb0[ch][:, off:off + w])
                                for k in range(1, KH):
                                    nc.vector.copy_predicated(
                                        root_acc[ch][k][:, off:off + w], mseg,
                                        hb12[ch][:, k - 1, off:off + w])
                        else:
                            if s == 0:
                                end_t[ch] = capp.tile([128, KH, C], BF16,
                                                      name=f"end{ch}",
                                                      tag=f"end{ch}")
                                nc.gpsimd.tensor_copy(end_t[ch][:, 0, :],
                                                      hb0[ch][:])
                                nc.gpsimd.tensor_copy(end_t[ch][:, 1:KH, :],
                                                      hb12[ch][:])
                            if s == L - 1:
                                start_t[ch] = capp.tile([128, KH, C], BF16,
                                                        name=f"start{ch}",
                                                        tag=f"start{ch}")
                                nc.gpsimd.tensor_copy(start_t[ch][:, 0, :],
                                                      hb0[ch][:])
                                nc.gpsimd.tensor_copy(start_t[ch][:, 1:KH, :],
                                                      hb12[ch][:])
                    if s + 1 < L:
                        for ch in range(NCH):
                            nd_cur[d, ch] = nd_nx[ch]

        if debug:
            for sp in range(2):
                for m in range(MT):
                    nc.sync.dma_start(dbg_span[sp, m], spanT[sp][m][:])
            for ch in range(NCH):
                for k in range(KH):
                    nc.sync.dma_start(dbg_racc[ch, k], root_acc[ch][k][:])
                nc.sync.dma_start(dbg_start[ch], start_t[ch][:])
                nc.sync.dma_start(dbg_end[ch], end_t[ch][:])

        # ---- phase 3: pair MLP -----------------------------------------
        with tc.tile_pool(name="mlpw", bufs=1) as mlpw, \
             tc.tile_pool(name="mlpp", bufs=4) as mlpp, \
             tc.tile_pool(name="mpsum", bufs=6, space="PSUM") as mpsum, \
             tc.tile_pool(name="pout", bufs=1, space="PSUM") as pout:
            w1_t = [loadc(mlpw, f"w1{k}", W1[k * 128:(k + 1) * 128, :],
                          [128, DEC_H], BF16) for k in range(3 * KH)]
            w2_t = [loadc(mlpw, f"w2{k}", W2[k * 128:(k + 1) * 128, :],
                          [128, DEC_OUT], BF16) for k in range(M4)]
            for ch in range(NCH):
                c0 = ch * C
                feats = ([root_acc[ch][k][:] for k in range(KH)]
                         + [start_t[ch][:, k, :] for k in range(KH)]
                         + [end_t[ch][:, k, :] for k in range(KH)])
                z_t = []
                for m in range(M4):
                    zp = mpsum.tile([128, C], FP32, name="zp2", tag="mm")
                    for k in range(3 * KH):
                        nc.tensor.matmul(zp[:], w1_t[k][:, m * 128:(m + 1) * 128],
                                         feats[k], start=(k == 0),
                                         stop=(k == 3 * KH - 1))
                    zs = mlpp.tile([128, C], BF16, name="zs", tag="zsum")
                    nc.vector.tensor_tensor(zs[:], zp[:], z_span[m][ch][:],
                                            ALU.add)
                    z = mlpp.tile([128, C], BF16, name="z", tag="z")
                    nc.scalar.activation(z[:], zs[:], ACTF.Tanh, bias=b1_t[m][:])
                    z_t.append(z)
                op = pout.tile([DEC_OUT, C], FP32, name="op", tag="op")
                for m in range(M4):
                    nc.tensor.matmul(op[:], w2_t[m][:], z_t[m][:], start=(m == 0),
                                     stop=(m == M4 - 1))
                osb = mlpp.tile([DEC_OUT, C], FP32, name="osb", tag="osb", bufs=2)
                nc.vector.tensor_scalar(osb[:], op[:], b2_t[:], None, ALU.add)
                nc.sync.dma_start(out_d[:, c0:c0 + C], osb[:])

    nc.compile()
    _dedupe_ldweights(nc)
    return nc


def _dedupe_ldweights(nc):
    """Remove PE InstLdweights whose weights AP equals the most recently
    retained one with only PE Matmults in between (the PE weight buffer is
    unchanged by other engines). Only wait-free/update-free loads are removed."""
    import concourse.mybir as _mb
    for name, bb in list(nc.bb_map.items()):
        insts = bb.bb.instructions
        out = []
        prev_sig = None
        removed = 0
        for inst in insts:
            tn = type(inst).__name__
            eng = getattr(inst, "engine", None)
            if eng == _mb.EngineType.PE:
                if tn == "InstLdweights":
                    si = inst.sync_info
                    clean = si is None or (not si.on_wait and not si.on_update)
                    try:
                        sig = str(inst.ins[0])
                    except Exception:
                        sig = None
                    if clean and sig is not None and sig == prev_sig:
                        removed += 1
                        continue
                    prev_sig = sig
                elif tn != "InstMatmult":
                    prev_sig = None
            out.append(inst)
        if removed:
            bb.bb.instructions = out


_CACHE = {}


def _get_program() -> bass.Bass:
    if "nc" not in _CACHE:
        _CACHE["nc"] = _build_program()
    return _CACHE["nc"]


def _prep_in_maps(inputs) -> list[dict]:
    f32 = np.float32
    node = np.asarray(inputs["node_embs"], f32)
    tokf = np.asarray(inputs["token_embs"], f32)
    rooti = np.asarray(inputs["root_idx"])

    # sort pairs by root_idx within each batch (pure layout permutation,
    # inverted on the output): enables the up-dir early-exit windows
    rb = rooti.reshape(B, PB)
    order = np.argsort(rb, axis=1, kind="stable")           # [B, PB]
    perm_full = (order + (np.arange(B) * PB)[:, None]).reshape(P)
    _CACHE["inv_perm"] = np.argsort(perm_full)
    node = node[perm_full]
    rooti = rooti[perm_full]
    rb_s = rooti.reshape(B, PB)
    # widths[s] = 256 - min over batches of #(root < s); exact cover for
    # every batch block on every core
    cnt = (rb_s[:, :, None] < np.arange(L)[None, None, :]).sum(1)   # [B, L]
    _CACHE["widths"] = tuple(int(256 - c) for c in cnt.min(0))

    def bperm(a):
        return np.take_along_axis(np.asarray(a), order, axis=1)

    inputs = dict(inputs,
                  p1_st=bperm(inputs["p1_st"]), p1_len=bperm(inputs["p1_len"]),
                  p2_st=bperm(inputs["p2_st"]), p2_len=bperm(inputs["p2_len"]))
    # [P, L, D] fp32 -> per-core [L, 128, 7, PS] fp8 with bias row appended:
    # rows 0..831 = x, row 832 = 1.0, rows 833..895 = 0 (7 k-subtiles of 128)
    n8 = node.astype(f8e4).reshape(NCORES, PS, L, D).transpose(0, 2, 3, 1)
    pad = np.zeros((NCORES, L, 896, PS), f8e4)
    pad[:, :, :D, :] = n8
    pad[:, :, D, :] = f8e4(1.0)
    node_sh = np.ascontiguousarray(
        pad.reshape(NCORES, L, 7, 128, PS).transpose(0, 1, 3, 2, 4))
    tok_sh = tokf.reshape(NCORES, NB, T, DT).astype(bf16)

    def span_arrays(st, ln):
        st = np.asarray(st).astype(f32)
        ln = np.asarray(ln).astype(f32)
        en = st + ln + 1.0
        rc = 1.0 / (ln + 1.0)
        return st, en, rc

    s1, e1, r1 = span_arrays(inputs["p1_st"], inputs["p1_len"])
    s2, e2, r2 = span_arrays(inputs["p2_st"], inputs["p2_len"])

    def pack_span(a1, a2):
        # [B, PB] x2 -> per-core [2, NB, PB]
        a = np.stack([a1, a2])  # [2, B, PB]
        return a.reshape(2, NCORES, NB, PB).transpose(1, 0, 2, 3)

    # [NCORES, 2, NB, 3, PB] fp16 rows: st | en | recip (values <= 503 are
    # exact in fp16; recip has 10-bit mantissa)
    spb = np.ascontiguousarray(np.stack(
        [pack_span(s1, s2), pack_span(e1, e2), pack_span(r1, r2)],
        axis=3).astype(np.float16)).reshape(NCORES, 2, NB, 3 * PB)

    # one-hot root masks, broadcast across partitions: [NCORES, L, 128, PS]
    oh = (rooti.reshape(NCORES, 1, PS) ==
          np.arange(L, dtype=rooti.dtype).reshape(1, L, 1))
    mask8_h = np.ascontiguousarray(np.broadcast_to(
        oh[:, :, None, :], (NCORES, L, 128, PS)).astype(np.uint8))

    # gate-column permutation i|o|u|f -> f|u|i|o
    perm = np.concatenate([np.arange(3 * H, 4 * H), np.arange(2 * H, 3 * H),
                           np.arange(0, H), np.arange(H, 2 * H)])

    def build_wdr(Wiou, Wf, Uiou, Uf, biou, bf):
        W = np.concatenate([np.asarray(Wiou, f32), np.asarray(Wf, f32)],
                           axis=1)[:, perm]      # [D, 4H]
        U = np.concatenate([np.asarray(Uiou, f32), np.asarray(Uf, f32)],
                           axis=1)[:, perm]      # [H, 4H]
        b = np.concatenate([np.asarray(biou, f32), np.asarray(bf, f32)])[perm]
        Wpad = np.zeros((NKB * 256, H4), f32)
        Wpad[:D] = W
        Wpad[D] = b
        Wpad[896:896 + H] = U
        W8 = (Wpad * WSC).astype(f8e4)
        # [5 kb, 2 grp, 128 p, H4] -> [128, 5, 2, H4]
        return np.ascontiguousarray(
            W8.reshape(NKB, 2, 128, H4).transpose(2, 0, 1, 3))

    Wu_h = build_wdr(inputs["Wiou_u"], inputs["Wf_u"], inputs["Uiou_u"],
                     inputs["Uf_u"], inputs["biou_u"], inputs["bf_u"])
    Wd_h = build_wdr(inputs["Wiou_d"], inputs["Wf_d"], inputs["Uiou_d"],
                     inputs["Uf_d"], inputs["biou_d"], inputs["bf_d"])
    W1_h = np.asarray(inputs["W1"], f32).astype(bf16)
    W2_h = np.asarray(inputs["W2"], f32).astype(bf16)
    b1_h = np.asarray(inputs["b1"], f32).reshape(M4, 128, 1)
    b2_h = np.asarray(inputs["b2"], f32).reshape(DEC_OUT, 1)
    ones_h = np.ones((1, 128), np.float16)
    iota_h = np.ascontiguousarray(
        (np.arange(T // 128, dtype=f32)[None, :] * 128
         + np.arange(128, dtype=f32)[:, None]))

    in_maps = []
    for c in range(NCORES):
        in_maps.append({
            "node_dr": node_sh[c], "tok": tok_sh[c],
            "spb": spb[c], "mask8_d": mask8_h[c],
            "Wu": Wu_h, "Wd": Wd_h,
            "W1": W1_h, "W2": W2_h,
            "b1": b1_h, "b2": b2_h, "ones": ones_h, "iota_c": iota_h,
        })
    return in_maps


def unpermute(out_full: np.ndarray) -> np.ndarray:
    """Invert the host-side pair sort on a full [P, ...] output."""
    return out_full[_CACHE["inv_perm"]]


def run(inputs, **kwargs):
    """Run on hardware; returns (output [P, DEC_OUT] fp32, BassKernelResults)."""
    in_maps = _prep_in_maps(inputs)
    nc = _get_program()
    res = run_bass_kernel_spmd(nc, in_maps, list(range(NCORES)), **kwargs)
    outs = [np.asarray(r["out"], np.float32).T for r in res.results]  # [PS, 7]
    return unpermute(np.concatenate(outs, axis=0)), res


def kernel(**inputs) -> np.ndarray:
    out, _ = run(inputs)
    return out

